# revision 35
# baseline (speedup 1.0000x reference)
"""Trainium2 Bass kernel: MultiHeadSelfAttention (B=2, S=2048, D=1024, H=16).

Self-contained. Accepts FULL inputs, returns FULL output.

Sharding (8 cores, SPMD, no collectives):
  core c -> batch b = c // 4, lane j = c % 4. Within a batch the 16 heads
  are sorted by valid_len (desc) and dealt round-robin to the 4 lanes, so
  slot i on every core holds a head from rank-quartet i. Each core computes
  q/k/v projections for its 4 heads, attention, and the row-parallel
  partial of the output projection (A @ Wo[:, heads].T, shape (S, D),
  written fp16). Host sums the 4 partials per batch.

The program is specialized to per-slot QUERY BUDGETS at 128 granularity:
budget[i] = ceil(max valid_len in rank-quartet i / 128) * 128. Query
chunks beyond a slot's budget are entirely masked rows, whose attention
output is exactly uniform (= mean of V), so they are filled from a
precomputed mean-V column instead of being computed. One program serves
all 8 cores; distinct budget tuples compile separately and are cached.

Device-side math notes:
  - All matmuls run in bf16 (fp32 PSUM accumulation). Score matmuls for
    two consecutive key-tiles run CONCURRENTLY on disjoint PE row-groups
    (the head's 64 k/q dims are duplicated into both partition halves).
  - Inputs land in single big SBUF tiles (x: [128,8,S], w: [128,8,256])
    shipped by the host in "SBUF image" layouts so every DMA reads
    fully-contiguous DRAM rows at full HBM bandwidth; x streams in column
    chunks across two queues so the first projections start ~5us after
    the DMA phase opens, and dummy matmuls hold the PE's activity-based
    clock governor at full rate until real work arrives.
  - The kernel is a global software pipeline: scores run one step ahead
    of exp+attn@V across all (chunk, head) sites; the first two sites run
    FUSED (their heads' score matmuls pair on disjoint PE row-groups
    straight out of k_sb/q_sb) so two exp streams fill the input-load
    window; q/k/v projections, per-token-tile output-projection units and
    everything else are emitted just-in-time inside the stream.
  - Queue discipline: norm chains own sync, khd/qd duplication and
    mid-kernel output writes own gpsimd, input loads spread over all
    three, tail output writes alternate sync/scalar.
  - The last two softmax norms use an all-on-chip path (1-lane DVE
    reciprocal + K=1 ones outer-product broadcast through PSUM) instead
    of the 4-hop DRAM re-partition bounce, so the final output-projection
    units are not gated on ~10us of DMA latency; the chunk gated by the
    second-to-last norm is deferred to the tail where it overlaps the
    last norm.
  - The reference masks ENTIRE query rows j >= valid_len to -1e6 before
    softmax, making those rows' attention exactly uniform (1/S each). For
    masked rows inside a computed chunk we multiply q by the row mask
    (shipped as uint8 [2,128,S]): masked query -> scores all 0 -> exp all
    1 -> uniform attention.
  - No max-subtraction in softmax: scores/8 are bounded (|s| < ~10), so
    exp() cannot overflow in fp32 and softmax is scale-invariant anyway.
  - Softmax denominator comes free from a ones-column appended to V
    (attn@V_aug yields sum(exp) in the extra output row).
  - mean-of-V is computed with 16 ones-stationary matmuls into a [1,260]
    PSUM row and re-partitioned via a 1KB DRAM bounce (not 64 N=1
    matmuls).
  - bq/bk/bv are zeros in this problem's setup_inputs. bv/bo are folded in
    EXACTLY on the host (rows of attn sum to 1, so attn@(v+bv) = attn@v+bv).
    If bq/bk were ever nonzero we fall back to a numpy reference path.
"""

import numpy as np

B, S, D = 2, 2048, 1024
H, DH = 16, 64
HPG = 4                 # heads per core
GW = HPG * DH           # 256
P = 128
N_CORES = 8
NCH = S // 512          # query chunk slots (512-aligned)
# x / k column chunks (the first is partition-split across two queues)
KCH = [(0, 512), (512, 512), (1024, 512), (1536, 512)]
T2C = [0, 0, 0, 0, 1, 1, 1, 1, 2, 2, 2, 2, 3, 3, 3, 3]  # key-tile -> KCH idx

_PROGS = {}             # budgets tuple -> compiled Bacc


def _to_bf16(a):
    import ml_dtypes
    return np.ascontiguousarray(np.asarray(a, dtype=np.float32)
                                .astype(ml_dtypes.bfloat16))


def _widths(budgets):
    """site widths[slot][i4] (0 = no site)."""
    return [[max(0, min(512, b - 512 * i4)) for i4 in range(NCH)]
            for b in budgets]


def _emit(tc, aps, budgets):
    """Emit the per-core program. budgets: 4 per-slot query budgets."""
    from contextlib import ExitStack

    import concourse.mybir as mybir

    nc = tc.nc
    f32 = mybir.dt.float32
    f16 = mybir.dt.float16
    bf16 = mybir.dt.bfloat16
    EXP = mybir.ActivationFunctionType.Exp
    COPY = mybir.ActivationFunctionType.Copy

    xT, wqT, wkT, wvT, woT, mask, out = (
        aps["xT"], aps["wqT"], aps["wkT"], aps["wvT"], aps["woT"],
        aps["mask"], aps["out"],
    )
    W = _widths(budgets)               # W[slot][i4]
    nchunks = [sum(1 for w in ws if w) for ws in W]   # sites per slot

    ctx = ExitStack()
    with ctx:
        sb = ctx.enter_context(tc.tile_pool(name="sb", bufs=1))
        # PSUM: scores get their own pool so projections/output-projection
        # matmuls never starve the exp pipeline.  4 + 2 + 2 = 8 banks.
        ps_s = ctx.enter_context(tc.tile_pool(name="ps_s", bufs=2,
                                              space="PSUM"))
        ps_p = ctx.enter_context(tc.tile_pool(name="ps_p", bufs=2,
                                              space="PSUM"))
        psav = ctx.enter_context(tc.tile_pool(name="psav", bufs=2,
                                              space="PSUM"))
        rot = ctx.enter_context(tc.tile_pool(name="rot", bufs=8))
        ost = ctx.enter_context(tc.tile_pool(name="ost", bufs=3))
        sml = ctx.enter_context(tc.tile_pool(name="sml", bufs=4))
        scr = ctx.enter_context(tc.tile_pool(name="scr", bufs=4, space="DRAM"))
        xw = ctx.enter_context(tc.tile_pool(name="xw", bufs=1))

        # persistent intermediates
        wo_sb = sb.tile([P, 2, D], bf16, name="wo")
        q_sb = [sb.tile([P, S], bf16, name=f"q{p}") for p in range(2)]
        k_sb = [sb.tile([P, S], bf16, name=f"k{p}") for p in range(2)]
        v_sb = [sb.tile([P, HPG, DH + 1], bf16, name=f"v{t}")
                for t in range(16)]
        a_sb = [sb.tile([P, S], bf16, name=f"a{c}") for c in range(2)]
        meanv = sb.tile([DH + 1, HPG], f32, name="meanv")
        ones = sb.tile([P, 1], bf16, name="ones")
        ones_row = sb.tile([1, 64], f32, name="ones_row")
        dummy = sb.tile([P, 512], bf16, name="dummy")
        # per-head k with the head's 64 dims duplicated into both partition
        # halves: lets two key-tiles' score matmuls run CONCURRENTLY on
        # disjoint PE row-groups (tile_position packing)
        khd = [sb.tile([P, S], bf16, name=f"khd{h}") for h in range(HPG)]
        qd_pool = ctx.enter_context(tc.tile_pool(name="qd", bufs=1))

        # ---- input loads ------------------------------------------------
        # big single tiles. The host ships x/w in "SBUF image" layouts so
        # every DMA reads fully-contiguous DRAM rows (4-8KB runs, full HBM
        # bandwidth); x streams in column chunks so the first projections
        # start as soon as the first ~1MB lands.
        x_sb = xw.tile([P, 8, S], bf16, name="x")
        wq_sb = xw.tile([P, 8, GW], bf16, name="wq")
        wk_sb = xw.tile([P, 8, GW], bf16, name="wk")
        wv_sb = xw.tile([P, 8, GW], bf16, name="wv")
        mk_sb = [xw.tile([P, S], mybir.dt.uint8, name=f"mk{p}") for p in range(2)]
        # warm the ACT exp table-set (~2.7us load) during the DMA phase
        # so the first real exp doesn't pay it on the critical path
        warm_in = sml.tile([1, 8], f32, name="warm_in")
        warm_out = sml.tile([1, 8], f32, name="warm_out")
        nc.any.memset(warm_in[:], 0.0)
        nc.scalar.activation(warm_out[:], warm_in[:], EXP,
                             bias=0.0, scale=0.125)
        nc.any.memset(ones[:], 1.0)
        nc.any.memset(ones_row[:], 1.0)
        nc.vector.memset(dummy[:], 0.0)

        def emit_dummies(n):
            # matmuls on a zero tile into the score-PSUM pool: pure PE
            # activity to hold the HAM clock governor at full rate while
            # real work is blocked on DMAs or norm latency
            for _ in range(n):
                pt = ps_s.tile([P, 1024], f32, name="ps_s")
                nc.tensor.matmul(pt[:, 0:512], dummy[:, 0:P], dummy[:],
                                 start=True, stop=True)

        def x_chunk(eng, ci, rows=slice(0, P)):
            c0, w = KCH[ci]
            eng.dma_start(
                x_sb[rows, :, c0:c0 + w],
                xT[rows, 8 * c0:8 * (c0 + w)].rearrange(
                    "p (d s) -> p d s", d=8))

        # Queue-balanced priority order. Per-queue DMA service is ~1/3 of
        # the ~330GB/s aggregate, so the gating inputs are split/spread:
        # x chunk0's partition halves ride two queues while wk/wq lead the
        # third; the fused first sites then stream behind the x chunks.
        x_chunk(nc.sync, 0, slice(0, 64))
        x_chunk(nc.gpsimd, 0, slice(64, P))
        nc.scalar.dma_start(wk_sb[:],
                            wkT[:].rearrange("p (d g) -> p d g", d=8))
        for pair in range(2):
            nc.gpsimd.dma_start(mk_sb[pair][:], mask[pair])
        nc.scalar.dma_start(wq_sb[:],
                            wqT[:].rearrange("p (d g) -> p d g", d=8))
        x_chunk(nc.sync, 1)
        nc.gpsimd.dma_start(wv_sb[:],
                            wvT[:].rearrange("p (d g) -> p d g", d=8))
        x_chunk(nc.scalar, 2)
        x_chunk(nc.sync, 3)
        nc.scalar.dma_start(wo_sb[:],
                            woT[:].rearrange("p (c e) -> p c e", c=2))

        # ---- projection emitters (called just-in-time) ------------------
        def emit_k_chunk(mt, ci):
            c0, w = KCH[ci]
            cs = slice(c0, c0 + w)
            pt = ps_p.tile([P, 512], f32, name="ps_p")[:, :w]
            for d in range(8):
                nc.tensor.matmul(
                    pt,
                    wk_sb[:, d, mt * P:(mt + 1) * P],
                    x_sb[:, d, cs],
                    start=(d == 0), stop=(d == 7),
                )
            nc.vector.tensor_copy(k_sb[mt][:, cs], pt)
            for rr in range(2):
                h = 2 * mt + rr
                src_ap = k_sb[mt][64 * rr:64 * rr + 64, cs]
                nc.gpsimd.dma_start(khd[h][0:64, cs], src_ap)
                nc.gpsimd.dma_start(khd[h][64:128, cs], src_ap)

        def emit_q_chunk(pair, i4, skip_dup=False):
            wq_w = max(W[2 * pair][i4], W[2 * pair + 1][i4])
            cs = slice(i4 * 512, i4 * 512 + wq_w)
            pt = ps_p.tile([P, 512], f32, name="ps_p")[:, :wq_w]
            for d in range(8):
                nc.tensor.matmul(
                    pt,
                    wq_sb[:, d, pair * P:(pair + 1) * P],
                    x_sb[:, d, cs],
                    start=(d == 0), stop=(d == 7),
                )
            # fold the row mask into q (masked query -> q = 0)
            nc.vector.tensor_mul(q_sb[pair][:, cs], pt, mk_sb[pair][:, cs])
            if skip_dup:
                return     # fused first sites read q_sb directly
            for rr in range(2):
                w_rr = W[2 * pair + rr][i4]
                if w_rr == 0:
                    continue
                qd = qd_pool.tile([P, wq_w], bf16, name=f"qd{pair}_{i4}_{rr}")
                qd_tiles[(pair, i4, rr)] = qd
                src_ap = q_sb[pair][64 * rr:64 * rr + 64,
                                    i4 * 512:i4 * 512 + w_rr]
                nc.gpsimd.dma_start(qd[0:64, :w_rr], src_ap)
                nc.gpsimd.dma_start(qd[64:128, :w_rr], src_ap)

        def emit_v_tile(t):
            pt = ps_p.tile([P, 512], f32, name="ps_p")[:, :GW]
            for d in range(8):
                nc.tensor.matmul(
                    pt,
                    x_sb[:, d, t * P:(t + 1) * P],
                    wv_sb[:, d, :],
                    start=(d == 0), stop=(d == 7),
                )
            nc.any.memset(v_sb[t][:], 1.0)   # ones column at [:, :, DH]
            nc.vector.tensor_copy(
                v_sb[t][:, :, 0:DH],
                pt.rearrange("p (h e) -> p h e", h=HPG),
            )

        def emit_meanv():
            # mean of V per head (= output of fully-masked query rows):
            # ones-stationary matmuls -> [1, (h,dh1)] row, then a 1KB DRAM
            # bounce re-partitions it to [dh1, h]
            pmv = ps_p.tile([P, 512], f32, name="ps_p")[0:1, :HPG * (DH + 1)]
            for jt in range(16):
                nc.tensor.matmul(
                    pmv,
                    ones[:],
                    v_sb[jt][:, :, :],
                    start=(jt == 0), stop=(jt == 15),
                )
            mvr = sml.tile([1, HPG * (DH + 1)], f32, name="mvr")
            nc.vector.tensor_scalar_mul(mvr[:], pmv, 1.0 / S)
            mvd = scr.tile([1, HPG * (DH + 1)], f32, name="mvd")
            nc.gpsimd.dma_start(mvd[:], mvr[:])
            nc.gpsimd.dma_start(
                meanv[:], mvd[:].rearrange("o (h e) -> (o e) h", h=HPG))

        def emit_fills():
            # fully-masked query ranges: attention output is exactly mean-V
            for pair in range(2):
                for rr in range(2):
                    h = 2 * pair + rr
                    b = budgets[h]
                    if b >= S:
                        continue
                    for i4 in range(b // 512, NCH):
                        lo = max(b, i4 * 512)
                        hi = (i4 + 1) * 512
                        if lo >= hi:
                            continue
                        nc.vector.tensor_copy(
                            a_sb[pair][64 * rr:64 * rr + 64, lo:hi],
                            meanv[0:DH, h:h + 1].to_broadcast((64, hi - lo)),
                        )

        def emit_final_t(i4, t4, tail=False, scalar_only=False,
                         out_eng=None):
            """Output projection for one 128-token tile of chunk i4 (fp16).

            Mid-kernel units run one t-tile at a time interleaved into the
            attention stream (so the PE queue never carries a 16-matmul
            block that starves the exp pipeline). Tail units borrow the
            freed score-PSUM pool: both 512-halves land in one 2-bank tile
            and the PSUM->fp16 copies run on vector and scalar in
            parallel. scalar_only keeps the DVE free for the concurrent
            norm chain. Output DMAs ride the otherwise-idle vector queue
            so they never delay the norm chains on sync/gpsimd.
            """
            t = i4 * 4 + t4
            out_eng = out_eng or nc.gpsimd
            ot = ost.tile([P, D], f16, name="ot")
            if tail:
                pf2 = ps_s.tile([P, 1024], f32, name="ps_s")
                for n2 in range(2):
                    for c in range(2):
                        nc.tensor.matmul(
                            pf2[:, n2 * 512:(n2 + 1) * 512],
                            a_sb[c][:, t * P:(t + 1) * P],
                            wo_sb[:, c, n2 * 512:(n2 + 1) * 512],
                            start=(c == 0), stop=(c == 1),
                        )
                if scalar_only:
                    nc.scalar.activation(ot[:, 0:512], pf2[:, 0:512], COPY)
                    nc.scalar.activation(ot[:, 512:1024], pf2[:, 512:1024],
                                         COPY)
                else:
                    nc.vector.tensor_copy(ot[:, 0:512], pf2[:, 0:512])
                    nc.scalar.activation(ot[:, 512:1024], pf2[:, 512:1024],
                                         COPY)
            else:
                for n2 in range(2):
                    pf = ps_p.tile([P, 512], f32, name="ps_p")
                    for c in range(2):
                        nc.tensor.matmul(
                            pf[:],
                            a_sb[c][:, t * P:(t + 1) * P],
                            wo_sb[:, c, n2 * 512:(n2 + 1) * 512],
                            start=(c == 0), stop=(c == 1),
                        )
                    nc.vector.tensor_copy(
                        ot[:, n2 * 512:(n2 + 1) * 512], pf[:])
            out_eng.dma_start(out[t * P:(t + 1) * P, :], ot[:])

        class Site:
            """One (chunk, pair, head-row) attention block, pipelined."""

            def __init__(self, i4, pair, rr):
                self.i4, self.pair, self.rr = i4, pair, rr
                self.h = 2 * pair + rr
                self.w = W[self.h][i4]
                self.rows = slice(64 * rr, 64 * rr + 64)
                self.qs = slice(i4 * 512, i4 * 512 + self.w)
                self.pav = psav.tile([DH + 1, 512], f32, name="psav")
                self.pses = []
                self.exs = {}

            def emit_scores(self, jtp, direct=False):
                # the jj=1 half always lands at column 512 so the matmul
                # output starts on a PSUM bank boundary (hardware requires
                # bank-aligned matmul destinations)
                w = self.w
                pse = ps_s.tile([P, 1024], f32, name="ps_s")
                if direct:
                    # head of the kernel: skip the khd/qd duplication DMAs
                    # (they sit on the critical path before the first exp)
                    for jj in range(2):
                        jt = jtp * 2 + jj
                        nc.tensor.matmul(
                            pse[:, jj * 512:jj * 512 + w],
                            k_sb[self.pair][self.rows,
                                            jt * P:(jt + 1) * P],
                            q_sb[self.pair][self.rows, self.qs],
                            start=True, stop=True,
                        )
                    self.pses.append(pse)
                    return
                # the two key-tiles use disjoint PE row-groups (partitions
                # 0-63 / 64-127 of the duplicated khd/qd tiles) and
                # different PSUM banks, so they execute concurrently
                qd = qd_tiles[(self.pair, self.i4, self.rr)]
                for jj in range(2):
                    jt = jtp * 2 + jj
                    half = slice(64 * jj, 64 * jj + 64)
                    # scores^T = k @ q^T for head h
                    nc.tensor.matmul(
                        pse[:, jj * 512:jj * 512 + w],
                        khd[self.h][half, jt * P:(jt + 1) * P],
                        qd[half, :w],
                        start=True, stop=True,
                    )
                self.pses.append(pse)

            def emit_exp(self, jtp):
                w = self.w
                ex = rot.tile([P, 1024], bf16, name="ex")
                self.exs[jtp] = ex
                if w == 512:
                    nc.scalar.activation(ex[:], self.pses[jtp][:],
                                         EXP, bias=0.0, scale=0.125)
                else:
                    for jj in range(2):
                        cs = slice(jj * 512, jj * 512 + w)
                        nc.scalar.activation(ex[:, cs], self.pses[jtp][:, cs],
                                             EXP, bias=0.0, scale=0.125)

            def emit_av(self, jtp):
                w = self.w
                ex = self.exs[jtp]
                for jj in range(2):
                    jt = jtp * 2 + jj
                    nc.tensor.matmul(
                        self.pav[:, :w],
                        v_sb[jt][:, self.h, :],
                        ex[:, jj * 512:jj * 512 + w],
                        start=(jtp == 0 and jj == 0),
                        stop=(jtp == 7 and jj == 1),
                    )

            def emit_exp_av(self, jtp):
                self.emit_exp(jtp)
                self.emit_av(jtp)

            def emit_norm(self, idx, last=False, on_chip=False):
                # softmax denominator -> reciprocal on 64 lanes via a DRAM
                # re-partition bounce (DMA cannot read SBUF with partition
                # step 0, and a 1-lane reciprocal costs 3.3us). on_chip
                # instead pays the slow 1-lane reciprocal and broadcasts it
                # with a K=1 ones outer-product matmul into PSUM -- no DMA
                # hops at all, for the latency-exposed final sites.
                w = self.w
                pav = self.pav
                if on_chip:
                    rc = sml.tile([1, 512], f32, name="rc")[:, :w]
                    nc.vector.tensor_copy(rc, pav[DH:DH + 1, :w])
                    rro = sml.tile([1, 512], f32, name="rro")[:, :w]
                    nc.vector.reciprocal(rro, rc)
                    rcp = ps_p.tile([P, 512], f32, name="ps_p")[0:DH, :w]
                    nc.tensor.matmul(rcp, ones_row[:], rro,
                                     start=True, stop=True)
                    # DVE can read only one PSUM operand per instruction
                    rcb = sml.tile([64, 512], f32, name="rcb")[:, :w]
                    nc.vector.tensor_copy(rcb, rcp)
                    nc.vector.tensor_mul(
                        a_sb[self.pair][self.rows, self.qs],
                        pav[0:DH, :w], rcb)
                    return
                eng = nc.gpsimd if last else nc.sync
                rc = sml.tile([1, 512], f32, name="rc")[:, :w]
                nc.vector.tensor_copy(rc, pav[DH:DH + 1, :w])
                sc = scr.tile([1, 512], f32, name="sc")[:, :w]
                eng.dma_start(sc, rc)
                rs = sml.tile([64, 8], f32, name="rs")[:, :w // 64]
                eng.dma_start(
                    rs, sc.rearrange("o (p j) -> (o p) j", p=64))
                rr_t = sml.tile([64, 8], f32, name="rr")[:, :w // 64]
                nc.vector.reciprocal(rr_t, rs)
                sc2 = scr.tile([1, 512], f32, name="sc2")[:, :w]
                eng.dma_start(
                    sc2.rearrange("o (p j) -> (o p) j", p=64), rr_t)
                rcb = sml.tile([64, 512], f32, name="rcb")[:, :w]
                eng.dma_start(rcb, sc2.partition_broadcast(64))
                nc.vector.tensor_mul(
                    a_sb[self.pair][self.rows, self.qs],
                    pav[0:DH, :w], rcb)

        # ---- stream schedule --------------------------------------------
        # pair 0 leads, pair 1 lags one chunk; q/k/v projections are
        # emitted just-in-time inside the stream; output-projection t-units
        # are drip-fed into the site steps once their chunk's last norm is
        # in flight. The earliest-ready chunk's units are HELD BACK to the
        # tail, where they fill the PE while the last site's norm chain
        # (4 serial DMA hops, ~6us latency) completes.
        site_items = []
        for i4 in range(NCH + 1):
            if i4 < NCH:
                for rr in range(2):
                    if W[rr][i4]:
                        site_items.append((i4, 0, rr))
            if 1 <= i4:
                for rr in range(2):
                    if W[2 + rr][i4 - 1]:
                        site_items.append((i4 - 1, 1, rr))
        last_pos = {}
        for idx, it in enumerate(site_items):
            last_pos[it[0]] = idx
        # chunk c's units are emittable one site after its last site (the
        # norm lags a site); fill-only chunks need meanv+fills (site 0).
        # Chunks ready exactly at the LAST site are deferred to the tail:
        # their gate norm completes during the last site's steps, so their
        # units cover the final norm chain's DMA latency on the PE.
        n_sites = len(site_items)
        ready_pos = {c: max(last_pos.get(c, -1) + 1, 1) for c in range(NCH)}
        tail1 = [c for c in range(NCH) if ready_pos[c] == n_sites - 1]
        tail2 = [c for c in range(NCH) if ready_pos[c] >= n_sites]

        emitted_q = set()
        qd_tiles = {}
        k_done = [0, 0]       # k chunks (KCH idx) emitted per pair
        v_done = [0]          # v tiles emitted so far

        def need_v(upto):
            while v_done[0] < min(upto, 16):
                emit_v_tile(v_done[0])
                v_done[0] += 1
            if v_done[0] == 16:
                v_done[0] = 17
                emit_meanv()
                emit_fills()

        def need_k(pair, upto):
            while k_done[pair] < min(upto, len(KCH)):
                emit_k_chunk(pair, k_done[pair])
                k_done[pair] += 1

        def prep_site(i4, pair):
            need_k(pair, len(KCH))
            if (pair, i4) not in emitted_q:
                emitted_q.add((pair, i4))
                emit_q_chunk(pair, i4)

        # prologue: first k/q chunks only; the rest stream in JIT
        fuse01 = (len(site_items) >= 2
                  and site_items[1][:2] == (site_items[0][0], 0))
        emit_dummies(14)
        need_k(0, 1)
        emitted_q.add((0, 0))
        emit_q_chunk(0, 0, skip_dup=fuse01)

        prev = None
        norm_idx = 0
        pend = []             # (i4, t4) final units ready to interleave

        def flush_prev():
            nonlocal prev, norm_idx
            if prev is not None:
                prev.emit_exp_av(7)
                # the last two norms are latency-exposed at the tail:
                # use the DMA-free on-chip path for them
                prev.emit_norm(norm_idx, on_chip=(norm_idx >= n_sites - 2))
                norm_idx += 1
                prev = None

        def emit_scores_fused(s0, s1, jtp):
            # both first sites read k_sb/q_sb directly on their own
            # 64-partition halves -> the two heads' score matmuls run
            # concurrently on disjoint PE row-groups without any khd/qd
            # duplication DMAs
            ps = [ps_s.tile([P, 1024], f32, name="ps_s") for _ in range(2)]
            for jj in range(2):
                jt = jtp * 2 + jj
                for s, pse in zip((s0, s1), ps):
                    nc.tensor.matmul(
                        pse[:, jj * 512:jj * 512 + s.w],
                        k_sb[s.pair][s.rows, jt * P:(jt + 1) * P],
                        q_sb[s.pair][s.rows, s.qs],
                        start=True, stop=True,
                    )
            s0.pses.append(ps[0])
            s1.pses.append(ps[1])

        for sidx, (i4, pair, rr) in enumerate(site_items):
            for c in range(NCH):
                if ready_pos[c] == sidx and c not in tail1:
                    pend.extend((c, t4) for t4 in range(4))
            if sidx == 1 and fuse01:
                continue
            if sidx == 0 and fuse01:
                # the first two sites (same chunk+pair, rr 0/1) run as a
                # FUSED stream paced by the x-chunk DMAs: two exps per
                # step keep ACT busy through the whole input-load window
                s0 = Site(*site_items[0])
                s1 = Site(*site_items[1])
                for jtp in range(8):
                    need_k(0, T2C[min(2 * jtp + 3, 15)] + 1)
                    emit_scores_fused(s0, s1, jtp)
                    s0.emit_exp(jtp)
                    s1.emit_exp(jtp)
                    need_v(2 * jtp + 2)
                    s0.emit_av(jtp)
                    s1.emit_av(jtp)
                    need_v(2 * jtp + 4)
                s0.emit_norm(norm_idx)
                s1.emit_norm(norm_idx + 1)
                norm_idx += 2
                if len(site_items) > 2:
                    ni4, npair, _ = site_items[2]
                    prep_site(ni4, npair)
                continue
            site = Site(i4, pair, rr)
            # make sure the NEXT site's inputs are also being produced
            # (not during an unfused site 0 - eager k chunks would queue
            # x-blocked matmuls ahead of its ready score work)
            if 0 < sidx and sidx + 1 < len(site_items):
                ni4, npair, _ = site_items[sidx + 1]
                prep_site(ni4, npair)
            for jtp in range(8):
                if sidx == 0:
                    # unfused fallback: x-DMA-paced single first site
                    need_k(0, T2C[min(2 * jtp + 3, 15)] + 1)
                    site.emit_scores(jtp, direct=True)
                    site.emit_exp(jtp)
                    need_v(2 * jtp + 2)
                    site.emit_av(jtp)
                    need_v(2 * jtp + 4)
                    continue
                site.emit_scores(jtp)
                if jtp == 0:
                    flush_prev()
                else:
                    site.emit_exp_av(jtp - 1)
                    if jtp % 2 == 1 and pend:
                        emit_final_t(*pend.pop(0))
            if sidx == 0:
                site.emit_norm(norm_idx)
                norm_idx += 1
            else:
                prev = site
        # tail: finish the last site's attn@V; the tail1 chunks' units
        # (gated by the SECOND-to-last norm, which completed during the
        # last site) keep the PE busy while the last norm chain's DMA hops
        # run on gpsimd; the last-norm-gated chunks close the kernel. Tail
        # output DMAs alternate sync/scalar so they trail in parallel and
        # never sit in front of the last chain's legs.
        t_eng = [nc.sync, nc.scalar]
        n_tail = 0
        if prev is not None:
            prev.emit_exp_av(7)
            emit_dummies(14)
            for c in tail1:
                pend.extend((c, t4) for t4 in range(4))
            for i4f, t4f in pend:
                emit_final_t(i4f, t4f, tail=True, scalar_only=True,
                             out_eng=t_eng[n_tail % 2])
                n_tail += 1
            pend = []
            prev.emit_norm(norm_idx, on_chip=True)
            norm_idx += 1
            prev = None
        need_v(16)
        for c in tail2:
            pend.extend((c, t4) for t4 in range(4))
        for i4f, t4f in pend:
            emit_final_t(i4f, t4f, tail=True, out_eng=t_eng[n_tail % 2])
            n_tail += 1


def build_program(budgets):
    """Build + schedule + compile the per-core program (cached per key)."""
    budgets = tuple(budgets)
    if budgets in _PROGS:
        return _PROGS[budgets]

    import concourse.mybir as mybir
    import concourse.tile as tile
    from concourse import bacc

    nc = bacc.Bacc("TRN2", target_bir_lowering=False, debug=False)
    f16 = mybir.dt.float16
    bf16 = mybir.dt.bfloat16
    # x/w ship in "SBUF image" layouts (see make_in_maps) so DMAs read
    # contiguous DRAM rows at full bandwidth
    aps = {
        "xT": nc.dram_tensor("xT", [P, 8 * S], bf16, kind="ExternalInput").ap(),
        "wqT": nc.dram_tensor("wqT", [P, 8 * GW], bf16,
                              kind="ExternalInput").ap(),
        "wkT": nc.dram_tensor("wkT", [P, 8 * GW], bf16,
                              kind="ExternalInput").ap(),
        "wvT": nc.dram_tensor("wvT", [P, 8 * GW], bf16,
                              kind="ExternalInput").ap(),
        "woT": nc.dram_tensor("woT", [P, 2 * D], bf16,
                              kind="ExternalInput").ap(),
        "mask": nc.dram_tensor("mask", [2, P, S], mybir.dt.uint8,
                               kind="ExternalInput").ap(),
        "out": nc.dram_tensor("out", [S, D], f16, kind="ExternalOutput").ap(),
    }
    with tile.TileContext(nc) as tc:
        _emit(tc, aps, budgets)
    nc.compile()
    _PROGS[budgets] = nc
    return nc


def plan(valid_lens):
    """Head->core assignment and the compile-time budget tuple.

    Returns (budgets, heads_per_core): heads_per_core[c] lists the 4
    global head indices (within core c's batch) in slot order. Budgets
    are 128-granular.
    """
    valid = np.asarray(valid_lens).reshape(B, H)
    heads_per_core = [None] * N_CORES
    quart_max = [0] * HPG
    for b in range(B):
        order = np.argsort(-valid[b], kind="stable")
        for j in range(HPG):
            hs = [int(order[4 * i + j]) for i in range(HPG)]
            heads_per_core[b * HPG + j] = hs
        for i in range(HPG):
            quart_max[i] = max(quart_max[i],
                               int(valid[b, order[4 * i]]))
    budgets = tuple(min(-(-m // 128) * 128, S) for m in quart_max)
    return budgets, heads_per_core


def _x_image(Xt_bf16):
    """[D, S] -> [128, 8*S] SBUF image: chunk-major, then (d, s) per row."""
    x8 = Xt_bf16.reshape(8, P, S)
    parts = [np.ascontiguousarray(
        x8[:, :, c0:c0 + w].transpose(1, 0, 2).reshape(P, 8 * w))
        for c0, w in KCH]
    return np.ascontiguousarray(np.concatenate(parts, axis=1))


def _w_image(Wt_bf16, groups):
    """[groups*128, F] -> [128, groups*F] SBUF image (d-major per row)."""
    g8 = Wt_bf16.reshape(groups, P, -1)
    return np.ascontiguousarray(
        g8.transpose(1, 0, 2).reshape(P, -1))


def make_in_maps(X, Wq, Wk, Wv, Wo, valid_lens):
    """Host-side sharding: build the 8 per-core input maps."""
    import ml_dtypes
    X = np.asarray(X, dtype=np.float32)
    valid = np.asarray(valid_lens).reshape(B, H)
    budgets, heads_per_core = plan(valid_lens)
    iota = np.arange(S)
    in_maps = []
    xTs = [_x_image(_to_bf16(X[b].T)) for b in range(B)]
    Wq, Wk, Wv, Wo = (np.asarray(a, np.float32) for a in (Wq, Wk, Wv, Wo))
    for c in range(N_CORES):
        b = c // HPG
        hs = heads_per_core[c]
        rows = np.concatenate([np.arange(h * DH, (h + 1) * DH) for h in hs])
        mask = np.empty((2, P, S), dtype=np.uint8)
        for p in range(2):
            for rr in range(2):
                h = hs[2 * p + rr]
                mask[p, 64 * rr:64 * rr + 64, :] = (
                    iota < int(valid[b, h])).astype(np.uint8)[None, :]
        in_maps.append({
            "xT": xTs[b],
            "wqT": _w_image(_to_bf16(Wq[rows, :].T), 8),
            "wkT": _w_image(_to_bf16(Wk[rows, :].T), 8),
            "wvT": _w_image(_to_bf16(Wv[rows, :].T), 8),
            "woT": _w_image(_to_bf16(Wo[:, rows].T), 2),
            "mask": mask,
        })
    return budgets, in_maps


def assemble(results, Wo, bv, bo):
    """Host-side unshard: sum row-parallel partials, fold bv/bo exactly."""
    out = np.zeros((B, S, D), dtype=np.float32)
    for c in range(N_CORES):
        b = c // HPG
        out[b] += np.asarray(results[c]["out"], dtype=np.float32)
    bias = (np.asarray(bv, np.float32) @ np.asarray(Wo, np.float32).T
            + np.asarray(bo, np.float32))
    out += bias[None, None, :]
    return out


def _numpy_fallback(X, Wq, bq, Wk, bk, Wv, bv, Wo, bo, valid_lens):
    X = np.asarray(X, np.float32)
    q = (X @ np.asarray(Wq, np.float32).T + np.asarray(bq, np.float32))
    k = (X @ np.asarray(Wk, np.float32).T + np.asarray(bk, np.float32))
    v = (X @ np.asarray(Wv, np.float32).T + np.asarray(bv, np.float32))

    def split(y):
        return (y.reshape(B, S, H, DH).transpose(0, 2, 1, 3)
                .reshape(B * H, S, DH))

    q, k, v = split(q), split(k), split(v)
    s = np.einsum("bqd,bkd->bqk", q, k) / np.sqrt(DH).astype(np.float32)
    rm = (np.arange(S)[None, :]
          < np.asarray(valid_lens).reshape(-1)[:, None])
    s = np.where(rm[:, :, None], s, -1e6)
    s = s - s.max(axis=-1, keepdims=True)
    e = np.exp(s)
    attn = e / e.sum(axis=-1, keepdims=True)
    o = np.einsum("bqk,bkd->bqd", attn, v)
    o = o.reshape(B, H, S, DH).transpose(0, 2, 1, 3).reshape(B, S, D)
    return o @ np.asarray(Wo, np.float32).T + np.asarray(bo, np.float32)


def run_cores(budgets, in_maps, trace=False, **kw):
    """Run the compiled program on cores 0-7."""
    from concourse.bass_utils import run_bass_kernel_spmd

    nc = build_program(budgets)
    return run_bass_kernel_spmd(nc, in_maps, list(range(N_CORES)),
                                trace=trace, **kw)


def kernel(X, Wq, bq, Wk, bk, Wv, bv, Wo, bo, valid_lens):
    if np.any(np.asarray(bq)) or np.any(np.asarray(bk)):
        # never the case for this problem's setup_inputs (zeros);
        # exact fallback kept for safety.
        return _numpy_fallback(X, Wq, bq, Wk, bk, Wv, bv, Wo, bo, valid_lens)
    budgets, in_maps = make_in_maps(X, Wq, Wk, Wv, Wo, valid_lens)
    res = run_cores(budgets, in_maps, trace=False)
    return assemble(res.results, Wo, bv, bo)


# revision 36
# speedup vs baseline: 1.0000x; 1.0000x over previous
"""Trainium2 Bass kernel: MultiHeadSelfAttention (B=2, S=2048, D=1024, H=16).

Self-contained. Accepts FULL inputs, returns FULL output.

Sharding (8 cores, SPMD, no collectives):
  core c -> batch b = c // 4, lane j = c % 4. Within a batch the 16 heads
  are sorted by valid_len (desc) and dealt round-robin to the 4 lanes, so
  slot i on every core holds a head from rank-quartet i. Each core computes
  q/k/v projections for its 4 heads, attention, and the row-parallel
  partial of the output projection (A @ Wo[:, heads].T, shape (S, D),
  written fp16). Host sums the 4 partials per batch.

The program is specialized to per-slot QUERY BUDGETS at 128 granularity:
budget[i] = ceil(max valid_len in rank-quartet i / 128) * 128. Query
chunks beyond a slot's budget are entirely masked rows, whose attention
output is exactly uniform (= mean of V), so they are filled from a
precomputed mean-V column instead of being computed. One program serves
all 8 cores; distinct budget tuples compile separately and are cached.

Device-side math notes:
  - All matmuls run in bf16 (fp32 PSUM accumulation). Score matmuls for
    two consecutive key-tiles run CONCURRENTLY on disjoint PE row-groups
    (the head's 64 k/q dims are duplicated into both partition halves).
  - Inputs land in single big SBUF tiles (x: [128,8,S], w: [128,8,256])
    shipped by the host in "SBUF image" layouts so every DMA reads
    fully-contiguous DRAM rows at full HBM bandwidth; x streams in column
    chunks across two queues so the first projections start ~5us after
    the DMA phase opens, and dummy matmuls hold the PE's activity-based
    clock governor at full rate until real work arrives.
  - The kernel is a global software pipeline: scores run one step ahead
    of exp+attn@V across all (chunk, head) sites; the first two sites run
    FUSED (their heads' score matmuls pair on disjoint PE row-groups
    straight out of k_sb/q_sb) so two exp streams fill the input-load
    window; q/k/v projections, per-token-tile output-projection units and
    everything else are emitted just-in-time inside the stream.
  - Queue discipline: norm chains own sync, khd/qd duplication and
    mid-kernel output writes own gpsimd, input loads spread over all
    three, tail output writes alternate sync/scalar.
  - The last two softmax norms use an all-on-chip path (1-lane DVE
    reciprocal + K=1 ones outer-product broadcast through PSUM) instead
    of the 4-hop DRAM re-partition bounce, so the final output-projection
    units are not gated on ~10us of DMA latency; the chunk gated by the
    second-to-last norm is deferred to the tail where it overlaps the
    last norm.
  - The reference masks ENTIRE query rows j >= valid_len to -1e6 before
    softmax, making those rows' attention exactly uniform (1/S each). For
    masked rows inside a computed chunk we multiply q by the row mask
    (shipped as uint8 [2,128,S]): masked query -> scores all 0 -> exp all
    1 -> uniform attention.
  - No max-subtraction in softmax: scores/8 are bounded (|s| < ~10), so
    exp() cannot overflow in fp32 and softmax is scale-invariant anyway.
  - Softmax denominator comes free from a ones-column appended to V
    (attn@V_aug yields sum(exp) in the extra output row).
  - mean-of-V is computed with 16 ones-stationary matmuls into a [1,260]
    PSUM row and re-partitioned via a 1KB DRAM bounce (not 64 N=1
    matmuls).
  - bq/bk/bv are zeros in this problem's setup_inputs. bv/bo are folded in
    EXACTLY on the host (rows of attn sum to 1, so attn@(v+bv) = attn@v+bv).
    If bq/bk were ever nonzero we fall back to a numpy reference path.
"""

import numpy as np

B, S, D = 2, 2048, 1024
H, DH = 16, 64
HPG = 4                 # heads per core
GW = HPG * DH           # 256
P = 128
N_CORES = 8
NCH = S // 512          # query chunk slots (512-aligned)
# x / k column chunks (the first is partition-split across two queues)
KCH = [(0, 512), (512, 512), (1024, 512), (1536, 512)]
T2C = [0, 0, 0, 0, 1, 1, 1, 1, 2, 2, 2, 2, 3, 3, 3, 3]  # key-tile -> KCH idx

_PROGS = {}             # budgets tuple -> compiled Bacc


def _to_bf16(a):
    import ml_dtypes
    return np.ascontiguousarray(np.asarray(a, dtype=np.float32)
                                .astype(ml_dtypes.bfloat16))


def _widths(budgets):
    """site widths[slot][i4] (0 = no site)."""
    return [[max(0, min(512, b - 512 * i4)) for i4 in range(NCH)]
            for b in budgets]


def _emit(tc, aps, budgets):
    """Emit the per-core program. budgets: 4 per-slot query budgets."""
    from contextlib import ExitStack

    import concourse.mybir as mybir

    nc = tc.nc
    f32 = mybir.dt.float32
    f16 = mybir.dt.float16
    bf16 = mybir.dt.bfloat16
    EXP = mybir.ActivationFunctionType.Exp
    COPY = mybir.ActivationFunctionType.Copy

    xT, wqT, wkT, wvT, woT, mask, out = (
        aps["xT"], aps["wqT"], aps["wkT"], aps["wvT"], aps["woT"],
        aps["mask"], aps["out"],
    )
    W = _widths(budgets)               # W[slot][i4]
    nchunks = [sum(1 for w in ws if w) for ws in W]   # sites per slot

    ctx = ExitStack()
    with ctx:
        sb = ctx.enter_context(tc.tile_pool(name="sb", bufs=1))
        # PSUM: scores get their own pool so projections/output-projection
        # matmuls never starve the exp pipeline.  4 + 2 + 2 = 8 banks.
        ps_s = ctx.enter_context(tc.tile_pool(name="ps_s", bufs=2,
                                              space="PSUM"))
        ps_p = ctx.enter_context(tc.tile_pool(name="ps_p", bufs=2,
                                              space="PSUM"))
        psav = ctx.enter_context(tc.tile_pool(name="psav", bufs=2,
                                              space="PSUM"))
        rot = ctx.enter_context(tc.tile_pool(name="rot", bufs=8))
        ost = ctx.enter_context(tc.tile_pool(name="ost", bufs=3))
        sml = ctx.enter_context(tc.tile_pool(name="sml", bufs=4))
        scr = ctx.enter_context(tc.tile_pool(name="scr", bufs=4, space="DRAM"))
        xw = ctx.enter_context(tc.tile_pool(name="xw", bufs=1))

        # persistent intermediates
        wo_sb = sb.tile([P, 2, D], bf16, name="wo")
        q_sb = [sb.tile([P, S], bf16, name=f"q{p}") for p in range(2)]
        k_sb = [sb.tile([P, S], bf16, name=f"k{p}") for p in range(2)]
        v_sb = [sb.tile([P, HPG, DH + 1], bf16, name=f"v{t}")
                for t in range(16)]
        a_sb = [sb.tile([P, S], bf16, name=f"a{c}") for c in range(2)]
        meanv = sb.tile([DH + 1, HPG], f32, name="meanv")
        ones = sb.tile([P, 1], bf16, name="ones")
        ones_row = sb.tile([1, 64], f32, name="ones_row")
        dummy = sb.tile([P, 512], bf16, name="dummy")
        # per-head k with the head's 64 dims duplicated into both partition
        # halves: lets two key-tiles' score matmuls run CONCURRENTLY on
        # disjoint PE row-groups (tile_position packing)
        khd = [sb.tile([P, S], bf16, name=f"khd{h}") for h in range(HPG)]
        qd_pool = ctx.enter_context(tc.tile_pool(name="qd", bufs=1))

        # ---- input loads ------------------------------------------------
        # big single tiles. The host ships x/w in "SBUF image" layouts so
        # every DMA reads fully-contiguous DRAM rows (4-8KB runs, full HBM
        # bandwidth); x streams in column chunks so the first projections
        # start as soon as the first ~1MB lands.
        x_sb = xw.tile([P, 8, S], bf16, name="x")
        wq_sb = xw.tile([P, 8, GW], bf16, name="wq")
        wk_sb = xw.tile([P, 8, GW], bf16, name="wk")
        wv_sb = xw.tile([P, 8, GW], bf16, name="wv")
        mk_sb = [xw.tile([P, S], mybir.dt.uint8, name=f"mk{p}") for p in range(2)]
        # warm the ACT exp table-set (~2.7us load) during the DMA phase
        # so the first real exp doesn't pay it on the critical path
        warm_in = sml.tile([1, 8], f32, name="warm_in")
        warm_out = sml.tile([1, 8], f32, name="warm_out")
        nc.any.memset(warm_in[:], 0.0)
        nc.scalar.activation(warm_out[:], warm_in[:], EXP,
                             bias=0.0, scale=0.125)
        nc.any.memset(ones[:], 1.0)
        nc.any.memset(ones_row[:], 1.0)
        nc.vector.memset(dummy[:], 0.0)

        def emit_dummies(n):
            # matmuls on a zero tile into the score-PSUM pool: pure PE
            # activity to hold the HAM clock governor at full rate while
            # real work is blocked on DMAs or norm latency
            for _ in range(n):
                pt = ps_s.tile([P, 1024], f32, name="ps_s")
                nc.tensor.matmul(pt[:, 0:512], dummy[:, 0:P], dummy[:],
                                 start=True, stop=True)

        def x_chunk(eng, ci, rows=slice(0, P)):
            c0, w = KCH[ci]
            eng.dma_start(
                x_sb[rows, :, c0:c0 + w],
                xT[rows, 8 * c0:8 * (c0 + w)].rearrange(
                    "p (d s) -> p d s", d=8))

        # Queue-balanced priority order. Per-queue DMA service is ~1/3 of
        # the ~330GB/s aggregate, so the gating inputs are split/spread:
        # x chunk0's partition halves ride two queues while wk/wq lead the
        # third; the fused first sites then stream behind the x chunks.
        x_chunk(nc.sync, 0, slice(0, 64))
        x_chunk(nc.gpsimd, 0, slice(64, P))
        nc.scalar.dma_start(wk_sb[:],
                            wkT[:].rearrange("p (d g) -> p d g", d=8))
        for pair in range(2):
            nc.gpsimd.dma_start(mk_sb[pair][:], mask[pair])
        nc.scalar.dma_start(wq_sb[:],
                            wqT[:].rearrange("p (d g) -> p d g", d=8))
        x_chunk(nc.sync, 1, slice(0, 64))
        x_chunk(nc.gpsimd, 1, slice(64, P))
        nc.scalar.dma_start(wv_sb[:],
                            wvT[:].rearrange("p (d g) -> p d g", d=8))
        x_chunk(nc.sync, 2, slice(0, 64))
        x_chunk(nc.gpsimd, 2, slice(64, P))
        x_chunk(nc.sync, 3, slice(0, 64))
        x_chunk(nc.gpsimd, 3, slice(64, P))
        nc.scalar.dma_start(wo_sb[:],
                            woT[:].rearrange("p (c e) -> p c e", c=2))

        # ---- projection emitters (called just-in-time) ------------------
        def emit_k_chunk(mt, ci):
            c0, w = KCH[ci]
            cs = slice(c0, c0 + w)
            pt = ps_p.tile([P, 512], f32, name="ps_p")[:, :w]
            for d in range(8):
                nc.tensor.matmul(
                    pt,
                    wk_sb[:, d, mt * P:(mt + 1) * P],
                    x_sb[:, d, cs],
                    start=(d == 0), stop=(d == 7),
                )
            nc.vector.tensor_copy(k_sb[mt][:, cs], pt)
            for rr in range(2):
                h = 2 * mt + rr
                src_ap = k_sb[mt][64 * rr:64 * rr + 64, cs]
                nc.gpsimd.dma_start(khd[h][0:64, cs], src_ap)
                nc.gpsimd.dma_start(khd[h][64:128, cs], src_ap)

        def emit_q_chunk(pair, i4, skip_dup=False):
            wq_w = max(W[2 * pair][i4], W[2 * pair + 1][i4])
            cs = slice(i4 * 512, i4 * 512 + wq_w)
            pt = ps_p.tile([P, 512], f32, name="ps_p")[:, :wq_w]
            for d in range(8):
                nc.tensor.matmul(
                    pt,
                    wq_sb[:, d, pair * P:(pair + 1) * P],
                    x_sb[:, d, cs],
                    start=(d == 0), stop=(d == 7),
                )
            # fold the row mask into q (masked query -> q = 0)
            nc.vector.tensor_mul(q_sb[pair][:, cs], pt, mk_sb[pair][:, cs])
            if skip_dup:
                return     # fused first sites read q_sb directly
            for rr in range(2):
                w_rr = W[2 * pair + rr][i4]
                if w_rr == 0:
                    continue
                qd = qd_pool.tile([P, wq_w], bf16, name=f"qd{pair}_{i4}_{rr}")
                qd_tiles[(pair, i4, rr)] = qd
                src_ap = q_sb[pair][64 * rr:64 * rr + 64,
                                    i4 * 512:i4 * 512 + w_rr]
                nc.gpsimd.dma_start(qd[0:64, :w_rr], src_ap)
                nc.gpsimd.dma_start(qd[64:128, :w_rr], src_ap)

        def emit_v_tile(t):
            pt = ps_p.tile([P, 512], f32, name="ps_p")[:, :GW]
            for d in range(8):
                nc.tensor.matmul(
                    pt,
                    x_sb[:, d, t * P:(t + 1) * P],
                    wv_sb[:, d, :],
                    start=(d == 0), stop=(d == 7),
                )
            nc.any.memset(v_sb[t][:], 1.0)   # ones column at [:, :, DH]
            nc.vector.tensor_copy(
                v_sb[t][:, :, 0:DH],
                pt.rearrange("p (h e) -> p h e", h=HPG),
            )

        def emit_meanv():
            # mean of V per head (= output of fully-masked query rows):
            # ones-stationary matmuls -> [1, (h,dh1)] row, then a 1KB DRAM
            # bounce re-partitions it to [dh1, h]
            pmv = ps_p.tile([P, 512], f32, name="ps_p")[0:1, :HPG * (DH + 1)]
            for jt in range(16):
                nc.tensor.matmul(
                    pmv,
                    ones[:],
                    v_sb[jt][:, :, :],
                    start=(jt == 0), stop=(jt == 15),
                )
            mvr = sml.tile([1, HPG * (DH + 1)], f32, name="mvr")
            nc.vector.tensor_scalar_mul(mvr[:], pmv, 1.0 / S)
            mvd = scr.tile([1, HPG * (DH + 1)], f32, name="mvd")
            nc.gpsimd.dma_start(mvd[:], mvr[:])
            nc.gpsimd.dma_start(
                meanv[:], mvd[:].rearrange("o (h e) -> (o e) h", h=HPG))

        def emit_fills():
            # fully-masked query ranges: attention output is exactly mean-V
            for pair in range(2):
                for rr in range(2):
                    h = 2 * pair + rr
                    b = budgets[h]
                    if b >= S:
                        continue
                    for i4 in range(b // 512, NCH):
                        lo = max(b, i4 * 512)
                        hi = (i4 + 1) * 512
                        if lo >= hi:
                            continue
                        nc.vector.tensor_copy(
                            a_sb[pair][64 * rr:64 * rr + 64, lo:hi],
                            meanv[0:DH, h:h + 1].to_broadcast((64, hi - lo)),
                        )

        def emit_final_t(i4, t4, tail=False, scalar_only=False,
                         out_eng=None):
            """Output projection for one 128-token tile of chunk i4 (fp16).

            Mid-kernel units run one t-tile at a time interleaved into the
            attention stream (so the PE queue never carries a 16-matmul
            block that starves the exp pipeline). Tail units borrow the
            freed score-PSUM pool: both 512-halves land in one 2-bank tile
            and the PSUM->fp16 copies run on vector and scalar in
            parallel. scalar_only keeps the DVE free for the concurrent
            norm chain. Output DMAs ride the otherwise-idle vector queue
            so they never delay the norm chains on sync/gpsimd.
            """
            t = i4 * 4 + t4
            out_eng = out_eng or nc.gpsimd
            ot = ost.tile([P, D], f16, name="ot")
            if tail:
                pf2 = ps_s.tile([P, 1024], f32, name="ps_s")
                for n2 in range(2):
                    for c in range(2):
                        nc.tensor.matmul(
                            pf2[:, n2 * 512:(n2 + 1) * 512],
                            a_sb[c][:, t * P:(t + 1) * P],
                            wo_sb[:, c, n2 * 512:(n2 + 1) * 512],
                            start=(c == 0), stop=(c == 1),
                        )
                if scalar_only:
                    nc.scalar.activation(ot[:, 0:512], pf2[:, 0:512], COPY)
                    nc.scalar.activation(ot[:, 512:1024], pf2[:, 512:1024],
                                         COPY)
                else:
                    nc.vector.tensor_copy(ot[:, 0:512], pf2[:, 0:512])
                    nc.scalar.activation(ot[:, 512:1024], pf2[:, 512:1024],
                                         COPY)
            else:
                for n2 in range(2):
                    pf = ps_p.tile([P, 512], f32, name="ps_p")
                    for c in range(2):
                        nc.tensor.matmul(
                            pf[:],
                            a_sb[c][:, t * P:(t + 1) * P],
                            wo_sb[:, c, n2 * 512:(n2 + 1) * 512],
                            start=(c == 0), stop=(c == 1),
                        )
                    nc.vector.tensor_copy(
                        ot[:, n2 * 512:(n2 + 1) * 512], pf[:])
            out_eng.dma_start(out[t * P:(t + 1) * P, :], ot[:])

        class Site:
            """One (chunk, pair, head-row) attention block, pipelined."""

            def __init__(self, i4, pair, rr):
                self.i4, self.pair, self.rr = i4, pair, rr
                self.h = 2 * pair + rr
                self.w = W[self.h][i4]
                self.rows = slice(64 * rr, 64 * rr + 64)
                self.qs = slice(i4 * 512, i4 * 512 + self.w)
                self.pav = psav.tile([DH + 1, 512], f32, name="psav")
                self.pses = []
                self.exs = {}

            def emit_scores(self, jtp, direct=False):
                # the jj=1 half always lands at column 512 so the matmul
                # output starts on a PSUM bank boundary (hardware requires
                # bank-aligned matmul destinations)
                w = self.w
                pse = ps_s.tile([P, 1024], f32, name="ps_s")
                if direct:
                    # head of the kernel: skip the khd/qd duplication DMAs
                    # (they sit on the critical path before the first exp)
                    for jj in range(2):
                        jt = jtp * 2 + jj
                        nc.tensor.matmul(
                            pse[:, jj * 512:jj * 512 + w],
                            k_sb[self.pair][self.rows,
                                            jt * P:(jt + 1) * P],
                            q_sb[self.pair][self.rows, self.qs],
                            start=True, stop=True,
                        )
                    self.pses.append(pse)
                    return
                # the two key-tiles use disjoint PE row-groups (partitions
                # 0-63 / 64-127 of the duplicated khd/qd tiles) and
                # different PSUM banks, so they execute concurrently
                qd = qd_tiles[(self.pair, self.i4, self.rr)]
                for jj in range(2):
                    jt = jtp * 2 + jj
                    half = slice(64 * jj, 64 * jj + 64)
                    # scores^T = k @ q^T for head h
                    nc.tensor.matmul(
                        pse[:, jj * 512:jj * 512 + w],
                        khd[self.h][half, jt * P:(jt + 1) * P],
                        qd[half, :w],
                        start=True, stop=True,
                    )
                self.pses.append(pse)

            def emit_exp(self, jtp):
                w = self.w
                ex = rot.tile([P, 1024], bf16, name="ex")
                self.exs[jtp] = ex
                if w == 512:
                    nc.scalar.activation(ex[:], self.pses[jtp][:],
                                         EXP, bias=0.0, scale=0.125)
                else:
                    for jj in range(2):
                        cs = slice(jj * 512, jj * 512 + w)
                        nc.scalar.activation(ex[:, cs], self.pses[jtp][:, cs],
                                             EXP, bias=0.0, scale=0.125)

            def emit_av(self, jtp):
                w = self.w
                ex = self.exs[jtp]
                for jj in range(2):
                    jt = jtp * 2 + jj
                    nc.tensor.matmul(
                        self.pav[:, :w],
                        v_sb[jt][:, self.h, :],
                        ex[:, jj * 512:jj * 512 + w],
                        start=(jtp == 0 and jj == 0),
                        stop=(jtp == 7 and jj == 1),
                    )

            def emit_exp_av(self, jtp):
                self.emit_exp(jtp)
                self.emit_av(jtp)

            def emit_norm(self, idx, last=False, on_chip=False):
                # softmax denominator -> reciprocal on 64 lanes via a DRAM
                # re-partition bounce (DMA cannot read SBUF with partition
                # step 0, and a 1-lane reciprocal costs 3.3us). on_chip
                # instead pays the slow 1-lane reciprocal and broadcasts it
                # with a K=1 ones outer-product matmul into PSUM -- no DMA
                # hops at all, for the latency-exposed final sites.
                w = self.w
                pav = self.pav
                if on_chip:
                    rc = sml.tile([1, 512], f32, name="rc")[:, :w]
                    nc.vector.tensor_copy(rc, pav[DH:DH + 1, :w])
                    rro = sml.tile([1, 512], f32, name="rro")[:, :w]
                    nc.vector.reciprocal(rro, rc)
                    rcp = ps_p.tile([P, 512], f32, name="ps_p")[0:DH, :w]
                    nc.tensor.matmul(rcp, ones_row[:], rro,
                                     start=True, stop=True)
                    # DVE can read only one PSUM operand per instruction
                    rcb = sml.tile([64, 512], f32, name="rcb")[:, :w]
                    nc.vector.tensor_copy(rcb, rcp)
                    nc.vector.tensor_mul(
                        a_sb[self.pair][self.rows, self.qs],
                        pav[0:DH, :w], rcb)
                    return
                eng = nc.gpsimd if last else nc.sync
                rc = sml.tile([1, 512], f32, name="rc")[:, :w]
                nc.vector.tensor_copy(rc, pav[DH:DH + 1, :w])
                sc = scr.tile([1, 512], f32, name="sc")[:, :w]
                eng.dma_start(sc, rc)
                rs = sml.tile([64, 8], f32, name="rs")[:, :w // 64]
                eng.dma_start(
                    rs, sc.rearrange("o (p j) -> (o p) j", p=64))
                rr_t = sml.tile([64, 8], f32, name="rr")[:, :w // 64]
                nc.vector.reciprocal(rr_t, rs)
                sc2 = scr.tile([1, 512], f32, name="sc2")[:, :w]
                eng.dma_start(
                    sc2.rearrange("o (p j) -> (o p) j", p=64), rr_t)
                rcb = sml.tile([64, 512], f32, name="rcb")[:, :w]
                eng.dma_start(rcb, sc2.partition_broadcast(64))
                nc.vector.tensor_mul(
                    a_sb[self.pair][self.rows, self.qs],
                    pav[0:DH, :w], rcb)

        # ---- stream schedule --------------------------------------------
        # pair 0 leads, pair 1 lags one chunk; q/k/v projections are
        # emitted just-in-time inside the stream; output-projection t-units
        # are drip-fed into the site steps once their chunk's last norm is
        # in flight. The earliest-ready chunk's units are HELD BACK to the
        # tail, where they fill the PE while the last site's norm chain
        # (4 serial DMA hops, ~6us latency) completes.
        site_items = []
        for i4 in range(NCH + 1):
            if i4 < NCH:
                for rr in range(2):
                    if W[rr][i4]:
                        site_items.append((i4, 0, rr))
            if 1 <= i4:
                for rr in range(2):
                    if W[2 + rr][i4 - 1]:
                        site_items.append((i4 - 1, 1, rr))
        last_pos = {}
        for idx, it in enumerate(site_items):
            last_pos[it[0]] = idx
        # chunk c's units are emittable one site after its last site (the
        # norm lags a site); fill-only chunks need meanv+fills (site 0).
        # Chunks ready exactly at the LAST site are deferred to the tail:
        # their gate norm completes during the last site's steps, so their
        # units cover the final norm chain's DMA latency on the PE.
        n_sites = len(site_items)
        ready_pos = {c: max(last_pos.get(c, -1) + 1, 1) for c in range(NCH)}
        tail1 = [c for c in range(NCH) if ready_pos[c] == n_sites - 1]
        tail2 = [c for c in range(NCH) if ready_pos[c] >= n_sites]

        emitted_q = set()
        qd_tiles = {}
        k_done = [0, 0]       # k chunks (KCH idx) emitted per pair
        v_done = [0]          # v tiles emitted so far

        def need_v(upto):
            while v_done[0] < min(upto, 16):
                emit_v_tile(v_done[0])
                v_done[0] += 1
            if v_done[0] == 16:
                v_done[0] = 17
                emit_meanv()
                emit_fills()

        def need_k(pair, upto):
            while k_done[pair] < min(upto, len(KCH)):
                emit_k_chunk(pair, k_done[pair])
                k_done[pair] += 1

        def prep_site(i4, pair):
            need_k(pair, len(KCH))
            if (pair, i4) not in emitted_q:
                emitted_q.add((pair, i4))
                emit_q_chunk(pair, i4)

        # prologue: first k/q chunks only; the rest stream in JIT
        fuse01 = (len(site_items) >= 2
                  and site_items[1][:2] == (site_items[0][0], 0))
        emit_dummies(14)
        need_k(0, 1)
        emitted_q.add((0, 0))
        emit_q_chunk(0, 0, skip_dup=fuse01)

        prev = None
        norm_idx = 0
        pend = []             # (i4, t4) final units ready to interleave

        def flush_prev():
            nonlocal prev, norm_idx
            if prev is not None:
                prev.emit_exp_av(7)
                # the last two norms are latency-exposed at the tail:
                # use the DMA-free on-chip path for them
                prev.emit_norm(norm_idx, on_chip=(norm_idx >= n_sites - 2))
                norm_idx += 1
                prev = None

        def emit_scores_fused(s0, s1, jtp):
            # both first sites read k_sb/q_sb directly on their own
            # 64-partition halves -> the two heads' score matmuls run
            # concurrently on disjoint PE row-groups without any khd/qd
            # duplication DMAs
            ps = [ps_s.tile([P, 1024], f32, name="ps_s") for _ in range(2)]
            for jj in range(2):
                jt = jtp * 2 + jj
                for s, pse in zip((s0, s1), ps):
                    nc.tensor.matmul(
                        pse[:, jj * 512:jj * 512 + s.w],
                        k_sb[s.pair][s.rows, jt * P:(jt + 1) * P],
                        q_sb[s.pair][s.rows, s.qs],
                        start=True, stop=True,
                    )
            s0.pses.append(ps[0])
            s1.pses.append(ps[1])

        for sidx, (i4, pair, rr) in enumerate(site_items):
            for c in range(NCH):
                if ready_pos[c] == sidx and c not in tail1:
                    pend.extend((c, t4) for t4 in range(4))
            if sidx == 1 and fuse01:
                continue
            if sidx == 0 and fuse01:
                # the first two sites (same chunk+pair, rr 0/1) run as a
                # FUSED stream paced by the x-chunk DMAs: two exps per
                # step keep ACT busy through the whole input-load window
                s0 = Site(*site_items[0])
                s1 = Site(*site_items[1])
                for jtp in range(8):
                    need_k(0, T2C[min(2 * jtp + 3, 15)] + 1)
                    emit_scores_fused(s0, s1, jtp)
                    s0.emit_exp(jtp)
                    s1.emit_exp(jtp)
                    need_v(2 * jtp + 2)
                    s0.emit_av(jtp)
                    s1.emit_av(jtp)
                    need_v(2 * jtp + 4)
                    if jtp == 4 and len(site_items) > 2:
                        # site 2's q chunk + duplication DMAs go out now,
                        # ahead of the khd-dup backlog on gpsimd, so its
                        # scores don't stall at the phase transition
                        ni4, npair, _ = site_items[2]
                        prep_site(ni4, npair)
                s0.emit_norm(norm_idx)
                s1.emit_norm(norm_idx + 1)
                norm_idx += 2
                if len(site_items) > 2:
                    ni4, npair, _ = site_items[2]
                    prep_site(ni4, npair)
                continue
            site = Site(i4, pair, rr)
            # make sure the NEXT site's inputs are also being produced
            # (not during an unfused site 0 - eager k chunks would queue
            # x-blocked matmuls ahead of its ready score work)
            if 0 < sidx and sidx + 1 < len(site_items):
                ni4, npair, _ = site_items[sidx + 1]
                prep_site(ni4, npair)
            for jtp in range(8):
                if sidx == 0:
                    # unfused fallback: x-DMA-paced single first site
                    need_k(0, T2C[min(2 * jtp + 3, 15)] + 1)
                    site.emit_scores(jtp, direct=True)
                    site.emit_exp(jtp)
                    need_v(2 * jtp + 2)
                    site.emit_av(jtp)
                    need_v(2 * jtp + 4)
                    continue
                site.emit_scores(jtp)
                if jtp == 0:
                    flush_prev()
                else:
                    site.emit_exp_av(jtp - 1)
                    if jtp % 2 == 1 and pend:
                        emit_final_t(*pend.pop(0))
            if sidx == 0:
                site.emit_norm(norm_idx)
                norm_idx += 1
            else:
                prev = site
        # tail: finish the last site's attn@V; the tail1 chunks' units
        # (gated by the SECOND-to-last norm, which completed during the
        # last site) keep the PE busy while the last norm chain's DMA hops
        # run on gpsimd; the last-norm-gated chunks close the kernel. Tail
        # output DMAs alternate sync/scalar so they trail in parallel and
        # never sit in front of the last chain's legs.
        t_eng = [nc.sync, nc.scalar]
        n_tail = 0
        if prev is not None:
            prev.emit_exp_av(7)
            emit_dummies(14)
            for c in tail1:
                pend.extend((c, t4) for t4 in range(4))
            for i4f, t4f in pend:
                emit_final_t(i4f, t4f, tail=True, scalar_only=True,
                             out_eng=t_eng[n_tail % 2])
                n_tail += 1
            pend = []
            prev.emit_norm(norm_idx, on_chip=True)
            norm_idx += 1
            prev = None
        need_v(16)
        for c in tail2:
            pend.extend((c, t4) for t4 in range(4))
        for i4f, t4f in pend:
            emit_final_t(i4f, t4f, tail=True, out_eng=t_eng[n_tail % 2])
            n_tail += 1


def build_program(budgets):
    """Build + schedule + compile the per-core program (cached per key)."""
    budgets = tuple(budgets)
    if budgets in _PROGS:
        return _PROGS[budgets]

    import concourse.mybir as mybir
    import concourse.tile as tile
    from concourse import bacc

    nc = bacc.Bacc("TRN2", target_bir_lowering=False, debug=False)
    f16 = mybir.dt.float16
    bf16 = mybir.dt.bfloat16
    # x/w ship in "SBUF image" layouts (see make_in_maps) so DMAs read
    # contiguous DRAM rows at full bandwidth
    aps = {
        "xT": nc.dram_tensor("xT", [P, 8 * S], bf16, kind="ExternalInput").ap(),
        "wqT": nc.dram_tensor("wqT", [P, 8 * GW], bf16,
                              kind="ExternalInput").ap(),
        "wkT": nc.dram_tensor("wkT", [P, 8 * GW], bf16,
                              kind="ExternalInput").ap(),
        "wvT": nc.dram_tensor("wvT", [P, 8 * GW], bf16,
                              kind="ExternalInput").ap(),
        "woT": nc.dram_tensor("woT", [P, 2 * D], bf16,
                              kind="ExternalInput").ap(),
        "mask": nc.dram_tensor("mask", [2, P, S], mybir.dt.uint8,
                               kind="ExternalInput").ap(),
        "out": nc.dram_tensor("out", [S, D], f16, kind="ExternalOutput").ap(),
    }
    with tile.TileContext(nc) as tc:
        _emit(tc, aps, budgets)
    nc.compile()
    _PROGS[budgets] = nc
    return nc


def plan(valid_lens):
    """Head->core assignment and the compile-time budget tuple.

    Returns (budgets, heads_per_core): heads_per_core[c] lists the 4
    global head indices (within core c's batch) in slot order. Budgets
    are 128-granular.
    """
    valid = np.asarray(valid_lens).reshape(B, H)
    heads_per_core = [None] * N_CORES
    quart_max = [0] * HPG
    for b in range(B):
        order = np.argsort(-valid[b], kind="stable")
        for j in range(HPG):
            hs = [int(order[4 * i + j]) for i in range(HPG)]
            heads_per_core[b * HPG + j] = hs
        for i in range(HPG):
            quart_max[i] = max(quart_max[i],
                               int(valid[b, order[4 * i]]))
    budgets = tuple(min(-(-m // 128) * 128, S) for m in quart_max)
    return budgets, heads_per_core


def _x_image(Xt_bf16):
    """[D, S] -> [128, 8*S] SBUF image: chunk-major, then (d, s) per row."""
    x8 = Xt_bf16.reshape(8, P, S)
    parts = [np.ascontiguousarray(
        x8[:, :, c0:c0 + w].transpose(1, 0, 2).reshape(P, 8 * w))
        for c0, w in KCH]
    return np.ascontiguousarray(np.concatenate(parts, axis=1))


def _w_image(Wt_bf16, groups):
    """[groups*128, F] -> [128, groups*F] SBUF image (d-major per row)."""
    g8 = Wt_bf16.reshape(groups, P, -1)
    return np.ascontiguousarray(
        g8.transpose(1, 0, 2).reshape(P, -1))


def make_in_maps(X, Wq, Wk, Wv, Wo, valid_lens):
    """Host-side sharding: build the 8 per-core input maps."""
    import ml_dtypes
    X = np.asarray(X, dtype=np.float32)
    valid = np.asarray(valid_lens).reshape(B, H)
    budgets, heads_per_core = plan(valid_lens)
    iota = np.arange(S)
    in_maps = []
    xTs = [_x_image(_to_bf16(X[b].T)) for b in range(B)]
    Wq, Wk, Wv, Wo = (np.asarray(a, np.float32) for a in (Wq, Wk, Wv, Wo))
    for c in range(N_CORES):
        b = c // HPG
        hs = heads_per_core[c]
        rows = np.concatenate([np.arange(h * DH, (h + 1) * DH) for h in hs])
        mask = np.empty((2, P, S), dtype=np.uint8)
        for p in range(2):
            for rr in range(2):
                h = hs[2 * p + rr]
                mask[p, 64 * rr:64 * rr + 64, :] = (
                    iota < int(valid[b, h])).astype(np.uint8)[None, :]
        in_maps.append({
            "xT": xTs[b],
            "wqT": _w_image(_to_bf16(Wq[rows, :].T), 8),
            "wkT": _w_image(_to_bf16(Wk[rows, :].T), 8),
            "wvT": _w_image(_to_bf16(Wv[rows, :].T), 8),
            "woT": _w_image(_to_bf16(Wo[:, rows].T), 2),
            "mask": mask,
        })
    return budgets, in_maps


def assemble(results, Wo, bv, bo):
    """Host-side unshard: sum row-parallel partials, fold bv/bo exactly."""
    out = np.zeros((B, S, D), dtype=np.float32)
    for c in range(N_CORES):
        b = c // HPG
        out[b] += np.asarray(results[c]["out"], dtype=np.float32)
    bias = (np.asarray(bv, np.float32) @ np.asarray(Wo, np.float32).T
            + np.asarray(bo, np.float32))
    out += bias[None, None, :]
    return out


def _numpy_fallback(X, Wq, bq, Wk, bk, Wv, bv, Wo, bo, valid_lens):
    X = np.asarray(X, np.float32)
    q = (X @ np.asarray(Wq, np.float32).T + np.asarray(bq, np.float32))
    k = (X @ np.asarray(Wk, np.float32).T + np.asarray(bk, np.float32))
    v = (X @ np.asarray(Wv, np.float32).T + np.asarray(bv, np.float32))

    def split(y):
        return (y.reshape(B, S, H, DH).transpose(0, 2, 1, 3)
                .reshape(B * H, S, DH))

    q, k, v = split(q), split(k), split(v)
    s = np.einsum("bqd,bkd->bqk", q, k) / np.sqrt(DH).astype(np.float32)
    rm = (np.arange(S)[None, :]
          < np.asarray(valid_lens).reshape(-1)[:, None])
    s = np.where(rm[:, :, None], s, -1e6)
    s = s - s.max(axis=-1, keepdims=True)
    e = np.exp(s)
    attn = e / e.sum(axis=-1, keepdims=True)
    o = np.einsum("bqk,bkd->bqd", attn, v)
    o = o.reshape(B, H, S, DH).transpose(0, 2, 1, 3).reshape(B, S, D)
    return o @ np.asarray(Wo, np.float32).T + np.asarray(bo, np.float32)


def run_cores(budgets, in_maps, trace=False, **kw):
    """Run the compiled program on cores 0-7."""
    from concourse.bass_utils import run_bass_kernel_spmd

    nc = build_program(budgets)
    return run_bass_kernel_spmd(nc, in_maps, list(range(N_CORES)),
                                trace=trace, **kw)


def kernel(X, Wq, bq, Wk, bk, Wv, bv, Wo, bo, valid_lens):
    if np.any(np.asarray(bq)) or np.any(np.asarray(bk)):
        # never the case for this problem's setup_inputs (zeros);
        # exact fallback kept for safety.
        return _numpy_fallback(X, Wq, bq, Wk, bk, Wv, bv, Wo, bo, valid_lens)
    budgets, in_maps = make_in_maps(X, Wq, Wk, Wv, Wo, valid_lens)
    res = run_cores(budgets, in_maps, trace=False)
    return assemble(res.results, Wo, bv, bo)


# revision 38
# speedup vs baseline: 1.0049x; 1.0049x over previous
"""Trainium2 Bass kernel: MultiHeadSelfAttention (B=2, S=2048, D=1024, H=16).

Self-contained. Accepts FULL inputs, returns FULL output.

Sharding (8 cores, SPMD, no collectives):
  core c -> batch b = c // 4, lane j = c % 4. Within a batch the 16 heads
  are sorted by valid_len (desc) and dealt round-robin to the 4 lanes, so
  slot i on every core holds a head from rank-quartet i. Each core computes
  q/k/v projections for its 4 heads, attention, and the row-parallel
  partial of the output projection (A @ Wo[:, heads].T, shape (S, D),
  written fp16). Host sums the 4 partials per batch.

The program is specialized to per-slot QUERY BUDGETS at 128 granularity:
budget[i] = ceil(max valid_len in rank-quartet i / 128) * 128. Query
chunks beyond a slot's budget are entirely masked rows, whose attention
output is exactly uniform (= mean of V), so they are filled from a
precomputed mean-V column instead of being computed. One program serves
all 8 cores; distinct budget tuples compile separately and are cached.

Device-side math notes:
  - All matmuls run in bf16 (fp32 PSUM accumulation). Score matmuls for
    two consecutive key-tiles run CONCURRENTLY on disjoint PE row-groups
    (the head's 64 k/q dims are duplicated into both partition halves).
  - Inputs land in single big SBUF tiles (x: [128,8,S], w: [128,8,256])
    shipped by the host in "SBUF image" layouts so every DMA reads
    fully-contiguous DRAM rows at full HBM bandwidth; x streams in column
    chunks across two queues so the first projections start ~5us after
    the DMA phase opens, and dummy matmuls hold the PE's activity-based
    clock governor at full rate until real work arrives.
  - The kernel is a global software pipeline: scores run one step ahead
    of exp+attn@V across all (chunk, head) sites; the first two sites run
    FUSED (their heads' score matmuls pair on disjoint PE row-groups
    straight out of k_sb/q_sb) so two exp streams fill the input-load
    window; q/k/v projections, per-token-tile output-projection units and
    everything else are emitted just-in-time inside the stream.
  - Queue discipline: norm chains own sync, khd/qd duplication and
    mid-kernel output writes own gpsimd, input loads spread over all
    three, tail output writes alternate sync/scalar.
  - The last two softmax norms use an all-on-chip path (1-lane DVE
    reciprocal + K=1 ones outer-product broadcast through PSUM) instead
    of the 4-hop DRAM re-partition bounce, so the final output-projection
    units are not gated on ~10us of DMA latency; the chunk gated by the
    second-to-last norm is deferred to the tail where it overlaps the
    last norm.
  - The reference masks ENTIRE query rows j >= valid_len to -1e6 before
    softmax, making those rows' attention exactly uniform (1/S each). For
    masked rows inside a computed chunk we multiply q by the row mask
    (shipped as uint8 [2,128,S]): masked query -> scores all 0 -> exp all
    1 -> uniform attention.
  - No max-subtraction in softmax: scores/8 are bounded (|s| < ~10), so
    exp() cannot overflow in fp32 and softmax is scale-invariant anyway.
  - Softmax denominator comes free from a ones-column appended to V
    (attn@V_aug yields sum(exp) in the extra output row).
  - mean-of-V is computed with 16 ones-stationary matmuls into a [1,260]
    PSUM row and re-partitioned via a 1KB DRAM bounce (not 64 N=1
    matmuls).
  - bq/bk/bv are zeros in this problem's setup_inputs. bv/bo are folded in
    EXACTLY on the host (rows of attn sum to 1, so attn@(v+bv) = attn@v+bv).
    If bq/bk were ever nonzero we fall back to a numpy reference path.
"""

import numpy as np

B, S, D = 2, 2048, 1024
H, DH = 16, 64
HPG = 4                 # heads per core
GW = HPG * DH           # 256
P = 128
N_CORES = 8
NCH = S // 512          # query chunk slots (512-aligned)
# x / k column chunks (the first is partition-split across two queues)
KCH = [(0, 512), (512, 512), (1024, 512), (1536, 512)]
T2C = [0, 0, 0, 0, 1, 1, 1, 1, 2, 2, 2, 2, 3, 3, 3, 3]  # key-tile -> KCH idx

_PROGS = {}             # budgets tuple -> compiled Bacc


def _to_bf16(a):
    import ml_dtypes
    return np.ascontiguousarray(np.asarray(a, dtype=np.float32)
                                .astype(ml_dtypes.bfloat16))


def _widths(budgets):
    """site widths[slot][i4] (0 = no site)."""
    return [[max(0, min(512, b - 512 * i4)) for i4 in range(NCH)]
            for b in budgets]


def _emit(tc, aps, budgets):
    """Emit the per-core program. budgets: 4 per-slot query budgets."""
    from contextlib import ExitStack

    import concourse.mybir as mybir

    nc = tc.nc
    f32 = mybir.dt.float32
    f16 = mybir.dt.float16
    bf16 = mybir.dt.bfloat16
    EXP = mybir.ActivationFunctionType.Exp
    COPY = mybir.ActivationFunctionType.Copy

    xT, wqT, wkT, wvT, woT, mask, out = (
        aps["xT"], aps["wqT"], aps["wkT"], aps["wvT"], aps["woT"],
        aps["mask"], aps["out"],
    )
    W = _widths(budgets)               # W[slot][i4]
    nchunks = [sum(1 for w in ws if w) for ws in W]   # sites per slot

    ctx = ExitStack()
    with ctx:
        sb = ctx.enter_context(tc.tile_pool(name="sb", bufs=1))
        # PSUM: scores get their own pool so projections/output-projection
        # matmuls never starve the exp pipeline.  4 + 2 + 2 = 8 banks.
        ps_s = ctx.enter_context(tc.tile_pool(name="ps_s", bufs=2,
                                              space="PSUM"))
        ps_p = ctx.enter_context(tc.tile_pool(name="ps_p", bufs=2,
                                              space="PSUM"))
        psav = ctx.enter_context(tc.tile_pool(name="psav", bufs=2,
                                              space="PSUM"))
        rot = ctx.enter_context(tc.tile_pool(name="rot", bufs=12))
        ost = ctx.enter_context(tc.tile_pool(name="ost", bufs=3))
        sml = ctx.enter_context(tc.tile_pool(name="sml", bufs=4))
        scr = ctx.enter_context(tc.tile_pool(name="scr", bufs=4, space="DRAM"))
        xw = ctx.enter_context(tc.tile_pool(name="xw", bufs=1))

        # persistent intermediates
        wo_sb = sb.tile([P, 2, D], bf16, name="wo")
        q_sb = [sb.tile([P, S], bf16, name=f"q{p}") for p in range(2)]
        k_sb = [sb.tile([P, S], bf16, name=f"k{p}") for p in range(2)]
        v_sb = [sb.tile([P, HPG, DH + 1], bf16, name=f"v{t}")
                for t in range(16)]
        a_sb = [sb.tile([P, S], bf16, name=f"a{c}") for c in range(2)]
        meanv = sb.tile([DH + 1, HPG], f32, name="meanv")
        ones = sb.tile([P, 1], bf16, name="ones")
        ones_row = sb.tile([1, 64], f32, name="ones_row")
        dummy = sb.tile([P, 512], bf16, name="dummy")
        # per-head k with the head's 64 dims duplicated into both partition
        # halves: lets two key-tiles' score matmuls run CONCURRENTLY on
        # disjoint PE row-groups (tile_position packing)
        khd = [sb.tile([P, S], bf16, name=f"khd{h}") for h in range(HPG)]
        qd_pool = ctx.enter_context(tc.tile_pool(name="qd", bufs=1))

        # ---- input loads ------------------------------------------------
        # big single tiles. The host ships x/w in "SBUF image" layouts so
        # every DMA reads fully-contiguous DRAM rows (4-8KB runs, full HBM
        # bandwidth); x streams in column chunks so the first projections
        # start as soon as the first ~1MB lands.
        x_sb = xw.tile([P, 8, S], bf16, name="x")
        wq_sb = xw.tile([P, 8, GW], bf16, name="wq")
        wk_sb = xw.tile([P, 8, GW], bf16, name="wk")
        wv_sb = xw.tile([P, 8, GW], bf16, name="wv")
        mk_sb = [xw.tile([P, S], mybir.dt.uint8, name=f"mk{p}") for p in range(2)]
        # warm the ACT exp table-set (~2.7us load) during the DMA phase
        # so the first real exp doesn't pay it on the critical path
        warm_in = sml.tile([1, 8], f32, name="warm_in")
        warm_out = sml.tile([1, 8], f32, name="warm_out")
        nc.any.memset(warm_in[:], 0.0)
        nc.scalar.activation(warm_out[:], warm_in[:], EXP,
                             bias=0.0, scale=0.125)
        nc.any.memset(ones[:], 1.0)
        nc.any.memset(ones_row[:], 1.0)
        nc.vector.memset(dummy[:], 0.0)

        def emit_dummies(n):
            # matmuls on a zero tile into the score-PSUM pool: pure PE
            # activity to hold the HAM clock governor at full rate while
            # real work is blocked on DMAs or norm latency
            for _ in range(n):
                pt = ps_s.tile([P, 1024], f32, name="ps_s")
                nc.tensor.matmul(pt[:, 0:512], dummy[:, 0:P], dummy[:],
                                 start=True, stop=True)

        def x_chunk(eng, ci, rows=slice(0, P)):
            c0, w = KCH[ci]
            eng.dma_start(
                x_sb[rows, :, c0:c0 + w],
                xT[rows, 8 * c0:8 * (c0 + w)].rearrange(
                    "p (d s) -> p d s", d=8))

        # Queue-balanced priority order. Per-queue DMA service is ~1/3 of
        # the ~330GB/s aggregate, so the gating inputs are split/spread:
        # x chunk0's partition halves ride two queues while wk/wq lead the
        # third; the fused first sites then stream behind the x chunks.
        x_chunk(nc.sync, 0, slice(0, 64))
        x_chunk(nc.gpsimd, 0, slice(64, P))
        nc.scalar.dma_start(wk_sb[:],
                            wkT[:].rearrange("p (d g) -> p d g", d=8))
        for pair in range(2):
            nc.gpsimd.dma_start(mk_sb[pair][:], mask[pair])
        nc.scalar.dma_start(wq_sb[:],
                            wqT[:].rearrange("p (d g) -> p d g", d=8))
        x_chunk(nc.sync, 1, slice(0, 64))
        x_chunk(nc.gpsimd, 1, slice(64, P))
        nc.scalar.dma_start(wv_sb[:],
                            wvT[:].rearrange("p (d g) -> p d g", d=8))
        x_chunk(nc.sync, 2, slice(0, 64))
        x_chunk(nc.gpsimd, 2, slice(64, P))
        x_chunk(nc.sync, 3, slice(0, 64))
        x_chunk(nc.gpsimd, 3, slice(64, P))
        nc.scalar.dma_start(wo_sb[:],
                            woT[:].rearrange("p (c e) -> p c e", c=2))

        # ---- projection emitters (called just-in-time) ------------------
        def emit_k_chunk(mt, ci):
            c0, w = KCH[ci]
            cs = slice(c0, c0 + w)
            pt = ps_p.tile([P, 512], f32, name="ps_p")[:, :w]
            for d in range(8):
                nc.tensor.matmul(
                    pt,
                    wk_sb[:, d, mt * P:(mt + 1) * P],
                    x_sb[:, d, cs],
                    start=(d == 0), stop=(d == 7),
                )
            nc.vector.tensor_copy(k_sb[mt][:, cs], pt)
            for rr in range(2):
                h = 2 * mt + rr
                src_ap = k_sb[mt][64 * rr:64 * rr + 64, cs]
                nc.gpsimd.dma_start(khd[h][0:64, cs], src_ap)
                nc.gpsimd.dma_start(khd[h][64:128, cs], src_ap)

        def emit_q_chunk(pair, i4, skip_dup=False):
            wq_w = max(W[2 * pair][i4], W[2 * pair + 1][i4])
            cs = slice(i4 * 512, i4 * 512 + wq_w)
            pt = ps_p.tile([P, 512], f32, name="ps_p")[:, :wq_w]
            for d in range(8):
                nc.tensor.matmul(
                    pt,
                    wq_sb[:, d, pair * P:(pair + 1) * P],
                    x_sb[:, d, cs],
                    start=(d == 0), stop=(d == 7),
                )
            # fold the row mask into q (masked query -> q = 0)
            nc.vector.tensor_mul(q_sb[pair][:, cs], pt, mk_sb[pair][:, cs])
            if skip_dup:
                return     # fused first sites read q_sb directly
            for rr in range(2):
                w_rr = W[2 * pair + rr][i4]
                if w_rr == 0:
                    continue
                qd = qd_pool.tile([P, wq_w], bf16, name=f"qd{pair}_{i4}_{rr}")
                qd_tiles[(pair, i4, rr)] = qd
                src_ap = q_sb[pair][64 * rr:64 * rr + 64,
                                    i4 * 512:i4 * 512 + w_rr]
                nc.gpsimd.dma_start(qd[0:64, :w_rr], src_ap)
                nc.gpsimd.dma_start(qd[64:128, :w_rr], src_ap)

        def emit_v_tile(t):
            pt = ps_p.tile([P, 512], f32, name="ps_p")[:, :GW]
            for d in range(8):
                nc.tensor.matmul(
                    pt,
                    x_sb[:, d, t * P:(t + 1) * P],
                    wv_sb[:, d, :],
                    start=(d == 0), stop=(d == 7),
                )
            nc.any.memset(v_sb[t][:], 1.0)   # ones column at [:, :, DH]
            nc.vector.tensor_copy(
                v_sb[t][:, :, 0:DH],
                pt.rearrange("p (h e) -> p h e", h=HPG),
            )

        def emit_meanv():
            # mean of V per head (= output of fully-masked query rows):
            # ones-stationary matmuls -> [1, (h,dh1)] row, then a 1KB DRAM
            # bounce re-partitions it to [dh1, h]
            pmv = ps_p.tile([P, 512], f32, name="ps_p")[0:1, :HPG * (DH + 1)]
            for jt in range(16):
                nc.tensor.matmul(
                    pmv,
                    ones[:],
                    v_sb[jt][:, :, :],
                    start=(jt == 0), stop=(jt == 15),
                )
            mvr = sml.tile([1, HPG * (DH + 1)], f32, name="mvr")
            nc.vector.tensor_scalar_mul(mvr[:], pmv, 1.0 / S)
            mvd = scr.tile([1, HPG * (DH + 1)], f32, name="mvd")
            nc.gpsimd.dma_start(mvd[:], mvr[:])
            nc.gpsimd.dma_start(
                meanv[:], mvd[:].rearrange("o (h e) -> (o e) h", h=HPG))

        def emit_fills():
            # fully-masked query ranges: attention output is exactly mean-V
            for pair in range(2):
                for rr in range(2):
                    h = 2 * pair + rr
                    b = budgets[h]
                    if b >= S:
                        continue
                    for i4 in range(b // 512, NCH):
                        lo = max(b, i4 * 512)
                        hi = (i4 + 1) * 512
                        if lo >= hi:
                            continue
                        nc.vector.tensor_copy(
                            a_sb[pair][64 * rr:64 * rr + 64, lo:hi],
                            meanv[0:DH, h:h + 1].to_broadcast((64, hi - lo)),
                        )

        def emit_final_t(i4, t4, tail=False, scalar_only=False,
                         out_eng=None):
            """Output projection for one 128-token tile of chunk i4 (fp16).

            Mid-kernel units run one t-tile at a time interleaved into the
            attention stream (so the PE queue never carries a 16-matmul
            block that starves the exp pipeline). Tail units borrow the
            freed score-PSUM pool: both 512-halves land in one 2-bank tile
            and the PSUM->fp16 copies run on vector and scalar in
            parallel. scalar_only keeps the DVE free for the concurrent
            norm chain. Output DMAs ride the otherwise-idle vector queue
            so they never delay the norm chains on sync/gpsimd.
            """
            t = i4 * 4 + t4
            out_eng = out_eng or nc.gpsimd
            ot = ost.tile([P, D], f16, name="ot")
            if tail:
                pf2 = ps_s.tile([P, 1024], f32, name="ps_s")
                for n2 in range(2):
                    for c in range(2):
                        nc.tensor.matmul(
                            pf2[:, n2 * 512:(n2 + 1) * 512],
                            a_sb[c][:, t * P:(t + 1) * P],
                            wo_sb[:, c, n2 * 512:(n2 + 1) * 512],
                            start=(c == 0), stop=(c == 1),
                        )
                if scalar_only:
                    nc.scalar.activation(ot[:, 0:512], pf2[:, 0:512], COPY)
                    nc.scalar.activation(ot[:, 512:1024], pf2[:, 512:1024],
                                         COPY)
                else:
                    nc.vector.tensor_copy(ot[:, 0:512], pf2[:, 0:512])
                    nc.scalar.activation(ot[:, 512:1024], pf2[:, 512:1024],
                                         COPY)
            else:
                for n2 in range(2):
                    pf = ps_p.tile([P, 512], f32, name="ps_p")
                    for c in range(2):
                        nc.tensor.matmul(
                            pf[:],
                            a_sb[c][:, t * P:(t + 1) * P],
                            wo_sb[:, c, n2 * 512:(n2 + 1) * 512],
                            start=(c == 0), stop=(c == 1),
                        )
                    nc.vector.tensor_copy(
                        ot[:, n2 * 512:(n2 + 1) * 512], pf[:])
            out_eng.dma_start(out[t * P:(t + 1) * P, :], ot[:])

        class Site:
            """One (chunk, pair, head-row) attention block, pipelined."""

            def __init__(self, i4, pair, rr):
                self.i4, self.pair, self.rr = i4, pair, rr
                self.h = 2 * pair + rr
                self.w = W[self.h][i4]
                self.rows = slice(64 * rr, 64 * rr + 64)
                self.qs = slice(i4 * 512, i4 * 512 + self.w)
                self.pav = psav.tile([DH + 1, 512], f32, name="psav")
                self.pses = []
                self.exs = {}

            def emit_scores(self, jtp, direct=False):
                # the jj=1 half always lands at column 512 so the matmul
                # output starts on a PSUM bank boundary (hardware requires
                # bank-aligned matmul destinations)
                w = self.w
                pse = ps_s.tile([P, 1024], f32, name="ps_s")
                if direct:
                    # head of the kernel: skip the khd/qd duplication DMAs
                    # (they sit on the critical path before the first exp)
                    for jj in range(2):
                        jt = jtp * 2 + jj
                        nc.tensor.matmul(
                            pse[:, jj * 512:jj * 512 + w],
                            k_sb[self.pair][self.rows,
                                            jt * P:(jt + 1) * P],
                            q_sb[self.pair][self.rows, self.qs],
                            start=True, stop=True,
                        )
                    self.pses.append(pse)
                    return
                # the two key-tiles use disjoint PE row-groups (partitions
                # 0-63 / 64-127 of the duplicated khd/qd tiles) and
                # different PSUM banks, so they execute concurrently
                qd = qd_tiles[(self.pair, self.i4, self.rr)]
                for jj in range(2):
                    jt = jtp * 2 + jj
                    half = slice(64 * jj, 64 * jj + 64)
                    # scores^T = k @ q^T for head h
                    nc.tensor.matmul(
                        pse[:, jj * 512:jj * 512 + w],
                        khd[self.h][half, jt * P:(jt + 1) * P],
                        qd[half, :w],
                        start=True, stop=True,
                    )
                self.pses.append(pse)

            def emit_exp(self, jtp):
                w = self.w
                ex = rot.tile([P, 1024], bf16, name="ex")
                self.exs[jtp] = ex
                if w == 512:
                    nc.scalar.activation(ex[:], self.pses[jtp][:],
                                         EXP, bias=0.0, scale=0.125)
                else:
                    for jj in range(2):
                        cs = slice(jj * 512, jj * 512 + w)
                        nc.scalar.activation(ex[:, cs], self.pses[jtp][:, cs],
                                             EXP, bias=0.0, scale=0.125)

            def emit_av(self, jtp):
                w = self.w
                ex = self.exs[jtp]
                for jj in range(2):
                    jt = jtp * 2 + jj
                    nc.tensor.matmul(
                        self.pav[:, :w],
                        v_sb[jt][:, self.h, :],
                        ex[:, jj * 512:jj * 512 + w],
                        start=(jtp == 0 and jj == 0),
                        stop=(jtp == 7 and jj == 1),
                    )

            def emit_exp_av(self, jtp):
                self.emit_exp(jtp)
                self.emit_av(jtp)

            def emit_norm(self, idx, last=False, on_chip=False):
                # softmax denominator -> reciprocal on 64 lanes via a DRAM
                # re-partition bounce (DMA cannot read SBUF with partition
                # step 0, and a 1-lane reciprocal costs 3.3us). on_chip
                # instead pays the slow 1-lane reciprocal and broadcasts it
                # with a K=1 ones outer-product matmul into PSUM -- no DMA
                # hops at all, for the latency-exposed final sites.
                w = self.w
                pav = self.pav
                if on_chip:
                    rc = sml.tile([1, 512], f32, name="rc")[:, :w]
                    nc.vector.tensor_copy(rc, pav[DH:DH + 1, :w])
                    rro = sml.tile([1, 512], f32, name="rro")[:, :w]
                    nc.vector.reciprocal(rro, rc)
                    rcp = ps_p.tile([P, 512], f32, name="ps_p")[0:DH, :w]
                    nc.tensor.matmul(rcp, ones_row[:], rro,
                                     start=True, stop=True)
                    # DVE can read only one PSUM operand per instruction
                    rcb = sml.tile([64, 512], f32, name="rcb")[:, :w]
                    nc.vector.tensor_copy(rcb, rcp)
                    nc.vector.tensor_mul(
                        a_sb[self.pair][self.rows, self.qs],
                        pav[0:DH, :w], rcb)
                    return
                eng = nc.gpsimd if last else nc.sync
                rc = sml.tile([1, 512], f32, name="rc")[:, :w]
                nc.vector.tensor_copy(rc, pav[DH:DH + 1, :w])
                sc = scr.tile([1, 512], f32, name="sc")[:, :w]
                eng.dma_start(sc, rc)
                rs = sml.tile([64, 8], f32, name="rs")[:, :w // 64]
                eng.dma_start(
                    rs, sc.rearrange("o (p j) -> (o p) j", p=64))
                rr_t = sml.tile([64, 8], f32, name="rr")[:, :w // 64]
                nc.vector.reciprocal(rr_t, rs)
                sc2 = scr.tile([1, 512], f32, name="sc2")[:, :w]
                eng.dma_start(
                    sc2.rearrange("o (p j) -> (o p) j", p=64), rr_t)
                rcb = sml.tile([64, 512], f32, name="rcb")[:, :w]
                eng.dma_start(rcb, sc2.partition_broadcast(64))
                nc.vector.tensor_mul(
                    a_sb[self.pair][self.rows, self.qs],
                    pav[0:DH, :w], rcb)

        # ---- stream schedule --------------------------------------------
        # pair 0 leads, pair 1 lags one chunk; q/k/v projections are
        # emitted just-in-time inside the stream; output-projection t-units
        # are drip-fed into the site steps once their chunk's last norm is
        # in flight. The earliest-ready chunk's units are HELD BACK to the
        # tail, where they fill the PE while the last site's norm chain
        # (4 serial DMA hops, ~6us latency) completes.
        site_items = []
        for i4 in range(NCH + 1):
            if i4 < NCH:
                for rr in range(2):
                    if W[rr][i4]:
                        site_items.append((i4, 0, rr))
            if 1 <= i4:
                for rr in range(2):
                    if W[2 + rr][i4 - 1]:
                        site_items.append((i4 - 1, 1, rr))
        last_pos = {}
        for idx, it in enumerate(site_items):
            last_pos[it[0]] = idx
        # chunk c's units are emittable one site after its last site (the
        # norm lags a site); fill-only chunks need meanv+fills (site 0).
        # Chunks ready exactly at the LAST site are deferred to the tail:
        # their gate norm completes during the last site's steps, so their
        # units cover the final norm chain's DMA latency on the PE.
        n_sites = len(site_items)
        ready_pos = {c: max(last_pos.get(c, -1) + 1, 1) for c in range(NCH)}
        tail1 = [c for c in range(NCH) if ready_pos[c] == n_sites - 1]
        tail2 = [c for c in range(NCH) if ready_pos[c] >= n_sites]

        emitted_q = set()
        qd_tiles = {}
        k_done = [0, 0]       # k chunks (KCH idx) emitted per pair
        v_done = [0]          # v tiles emitted so far

        def need_v(upto):
            while v_done[0] < min(upto, 16):
                emit_v_tile(v_done[0])
                v_done[0] += 1
            if v_done[0] == 16:
                v_done[0] = 17
                emit_meanv()
                emit_fills()

        def need_k(pair, upto):
            while k_done[pair] < min(upto, len(KCH)):
                emit_k_chunk(pair, k_done[pair])
                k_done[pair] += 1

        def prep_site(i4, pair):
            need_k(pair, len(KCH))
            if (pair, i4) not in emitted_q:
                emitted_q.add((pair, i4))
                emit_q_chunk(pair, i4)

        # prologue: first k/q chunks only; the rest stream in JIT
        fuse01 = (len(site_items) >= 2
                  and site_items[1][:2] == (site_items[0][0], 0))
        emit_dummies(14)
        need_k(0, 1)
        emitted_q.add((0, 0))
        emit_q_chunk(0, 0, skip_dup=fuse01)

        prev = None
        norm_idx = 0
        pend = []             # (i4, t4) final units ready to interleave

        def flush_prev():
            nonlocal prev, norm_idx
            if prev is not None:
                prev.emit_exp_av(7)
                # the last two norms are latency-exposed at the tail:
                # use the DMA-free on-chip path for them
                prev.emit_norm(norm_idx, on_chip=(norm_idx >= n_sites - 2))
                norm_idx += 1
                prev = None

        def emit_scores_fused(s0, s1, jtp):
            # both first sites read k_sb/q_sb directly on their own
            # 64-partition halves -> the two heads' score matmuls run
            # concurrently on disjoint PE row-groups without any khd/qd
            # duplication DMAs
            ps = [ps_s.tile([P, 1024], f32, name="ps_s") for _ in range(2)]
            for jj in range(2):
                jt = jtp * 2 + jj
                for s, pse in zip((s0, s1), ps):
                    nc.tensor.matmul(
                        pse[:, jj * 512:jj * 512 + s.w],
                        k_sb[s.pair][s.rows, jt * P:(jt + 1) * P],
                        q_sb[s.pair][s.rows, s.qs],
                        start=True, stop=True,
                    )
            s0.pses.append(ps[0])
            s1.pses.append(ps[1])

        for sidx, (i4, pair, rr) in enumerate(site_items):
            for c in range(NCH):
                if ready_pos[c] == sidx and c not in tail1:
                    pend.extend((c, t4) for t4 in range(4))
            if sidx == 1 and fuse01:
                continue
            if sidx == 0 and fuse01:
                # the first two sites (same chunk+pair, rr 0/1) run as a
                # FUSED stream paced by the x-chunk DMAs: two exps per
                # step keep ACT busy through the whole input-load window
                s0 = Site(*site_items[0])
                s1 = Site(*site_items[1])
                for jtp in range(8):
                    need_k(0, T2C[min(2 * jtp + 3, 15)] + 1)
                    emit_scores_fused(s0, s1, jtp)
                    s0.emit_exp(jtp)
                    s1.emit_exp(jtp)
                    need_v(2 * jtp + 2)
                    s0.emit_av(jtp)
                    s1.emit_av(jtp)
                    need_v(2 * jtp + 4)
                s0.emit_norm(norm_idx)
                s1.emit_norm(norm_idx + 1)
                norm_idx += 2
                if len(site_items) > 2:
                    ni4, npair, _ = site_items[2]
                    prep_site(ni4, npair)
                continue
            site = Site(i4, pair, rr)
            # make sure the NEXT site's inputs are also being produced
            # (not during an unfused site 0 - eager k chunks would queue
            # x-blocked matmuls ahead of its ready score work)
            if 0 < sidx and sidx + 1 < len(site_items):
                ni4, npair, _ = site_items[sidx + 1]
                prep_site(ni4, npair)
            for jtp in range(8):
                if sidx == 0:
                    # unfused fallback: x-DMA-paced single first site
                    need_k(0, T2C[min(2 * jtp + 3, 15)] + 1)
                    site.emit_scores(jtp, direct=True)
                    site.emit_exp(jtp)
                    need_v(2 * jtp + 2)
                    site.emit_av(jtp)
                    need_v(2 * jtp + 4)
                    continue
                site.emit_scores(jtp)
                if jtp == 0:
                    flush_prev()
                else:
                    site.emit_exp_av(jtp - 1)
                    if jtp % 2 == 1 and pend:
                        emit_final_t(*pend.pop(0))
            if sidx == 0:
                site.emit_norm(norm_idx)
                norm_idx += 1
            else:
                prev = site
        # tail: finish the last site's attn@V; the tail1 chunks' units
        # (gated by the SECOND-to-last norm, which completed during the
        # last site) keep the PE busy while the last norm chain's DMA hops
        # run on gpsimd; the last-norm-gated chunks close the kernel. Tail
        # output DMAs alternate sync/scalar so they trail in parallel and
        # never sit in front of the last chain's legs.
        t_eng = [nc.sync, nc.scalar]
        n_tail = 0
        if prev is not None:
            prev.emit_exp_av(7)
            emit_dummies(14)
            for c in tail1:
                pend.extend((c, t4) for t4 in range(4))
            for i4f, t4f in pend:
                emit_final_t(i4f, t4f, tail=True, scalar_only=True,
                             out_eng=t_eng[n_tail % 2])
                n_tail += 1
            pend = []
            prev.emit_norm(norm_idx, on_chip=True)
            norm_idx += 1
            prev = None
        need_v(16)
        for c in tail2:
            pend.extend((c, t4) for t4 in range(4))
        for i4f, t4f in pend:
            emit_final_t(i4f, t4f, tail=True, out_eng=t_eng[n_tail % 2])
            n_tail += 1


def build_program(budgets):
    """Build + schedule + compile the per-core program (cached per key)."""
    budgets = tuple(budgets)
    if budgets in _PROGS:
        return _PROGS[budgets]

    import concourse.mybir as mybir
    import concourse.tile as tile
    from concourse import bacc

    nc = bacc.Bacc("TRN2", target_bir_lowering=False, debug=False)
    f16 = mybir.dt.float16
    bf16 = mybir.dt.bfloat16
    # x/w ship in "SBUF image" layouts (see make_in_maps) so DMAs read
    # contiguous DRAM rows at full bandwidth
    aps = {
        "xT": nc.dram_tensor("xT", [P, 8 * S], bf16, kind="ExternalInput").ap(),
        "wqT": nc.dram_tensor("wqT", [P, 8 * GW], bf16,
                              kind="ExternalInput").ap(),
        "wkT": nc.dram_tensor("wkT", [P, 8 * GW], bf16,
                              kind="ExternalInput").ap(),
        "wvT": nc.dram_tensor("wvT", [P, 8 * GW], bf16,
                              kind="ExternalInput").ap(),
        "woT": nc.dram_tensor("woT", [P, 2 * D], bf16,
                              kind="ExternalInput").ap(),
        "mask": nc.dram_tensor("mask", [2, P, S], mybir.dt.uint8,
                               kind="ExternalInput").ap(),
        "out": nc.dram_tensor("out", [S, D], f16, kind="ExternalOutput").ap(),
    }
    with tile.TileContext(nc) as tc:
        _emit(tc, aps, budgets)
    nc.compile()
    _PROGS[budgets] = nc
    return nc


def plan(valid_lens):
    """Head->core assignment and the compile-time budget tuple.

    Returns (budgets, heads_per_core): heads_per_core[c] lists the 4
    global head indices (within core c's batch) in slot order. Budgets
    are 128-granular.
    """
    valid = np.asarray(valid_lens).reshape(B, H)
    heads_per_core = [None] * N_CORES
    quart_max = [0] * HPG
    for b in range(B):
        order = np.argsort(-valid[b], kind="stable")
        for j in range(HPG):
            hs = [int(order[4 * i + j]) for i in range(HPG)]
            heads_per_core[b * HPG + j] = hs
        for i in range(HPG):
            quart_max[i] = max(quart_max[i],
                               int(valid[b, order[4 * i]]))
    budgets = tuple(min(-(-m // 128) * 128, S) for m in quart_max)
    return budgets, heads_per_core


def _x_image(Xt_bf16):
    """[D, S] -> [128, 8*S] SBUF image: chunk-major, then (d, s) per row."""
    x8 = Xt_bf16.reshape(8, P, S)
    parts = [np.ascontiguousarray(
        x8[:, :, c0:c0 + w].transpose(1, 0, 2).reshape(P, 8 * w))
        for c0, w in KCH]
    return np.ascontiguousarray(np.concatenate(parts, axis=1))


def _w_image(Wt_bf16, groups):
    """[groups*128, F] -> [128, groups*F] SBUF image (d-major per row)."""
    g8 = Wt_bf16.reshape(groups, P, -1)
    return np.ascontiguousarray(
        g8.transpose(1, 0, 2).reshape(P, -1))


def make_in_maps(X, Wq, Wk, Wv, Wo, valid_lens):
    """Host-side sharding: build the 8 per-core input maps."""
    import ml_dtypes
    X = np.asarray(X, dtype=np.float32)
    valid = np.asarray(valid_lens).reshape(B, H)
    budgets, heads_per_core = plan(valid_lens)
    iota = np.arange(S)
    in_maps = []
    xTs = [_x_image(_to_bf16(X[b].T)) for b in range(B)]
    Wq, Wk, Wv, Wo = (np.asarray(a, np.float32) for a in (Wq, Wk, Wv, Wo))
    for c in range(N_CORES):
        b = c // HPG
        hs = heads_per_core[c]
        rows = np.concatenate([np.arange(h * DH, (h + 1) * DH) for h in hs])
        mask = np.empty((2, P, S), dtype=np.uint8)
        for p in range(2):
            for rr in range(2):
                h = hs[2 * p + rr]
                mask[p, 64 * rr:64 * rr + 64, :] = (
                    iota < int(valid[b, h])).astype(np.uint8)[None, :]
        in_maps.append({
            "xT": xTs[b],
            "wqT": _w_image(_to_bf16(Wq[rows, :].T), 8),
            "wkT": _w_image(_to_bf16(Wk[rows, :].T), 8),
            "wvT": _w_image(_to_bf16(Wv[rows, :].T), 8),
            "woT": _w_image(_to_bf16(Wo[:, rows].T), 2),
            "mask": mask,
        })
    return budgets, in_maps


def assemble(results, Wo, bv, bo):
    """Host-side unshard: sum row-parallel partials, fold bv/bo exactly."""
    out = np.zeros((B, S, D), dtype=np.float32)
    for c in range(N_CORES):
        b = c // HPG
        out[b] += np.asarray(results[c]["out"], dtype=np.float32)
    bias = (np.asarray(bv, np.float32) @ np.asarray(Wo, np.float32).T
            + np.asarray(bo, np.float32))
    out += bias[None, None, :]
    return out


def _numpy_fallback(X, Wq, bq, Wk, bk, Wv, bv, Wo, bo, valid_lens):
    X = np.asarray(X, np.float32)
    q = (X @ np.asarray(Wq, np.float32).T + np.asarray(bq, np.float32))
    k = (X @ np.asarray(Wk, np.float32).T + np.asarray(bk, np.float32))
    v = (X @ np.asarray(Wv, np.float32).T + np.asarray(bv, np.float32))

    def split(y):
        return (y.reshape(B, S, H, DH).transpose(0, 2, 1, 3)
                .reshape(B * H, S, DH))

    q, k, v = split(q), split(k), split(v)
    s = np.einsum("bqd,bkd->bqk", q, k) / np.sqrt(DH).astype(np.float32)
    rm = (np.arange(S)[None, :]
          < np.asarray(valid_lens).reshape(-1)[:, None])
    s = np.where(rm[:, :, None], s, -1e6)
    s = s - s.max(axis=-1, keepdims=True)
    e = np.exp(s)
    attn = e / e.sum(axis=-1, keepdims=True)
    o = np.einsum("bqk,bkd->bqd", attn, v)
    o = o.reshape(B, H, S, DH).transpose(0, 2, 1, 3).reshape(B, S, D)
    return o @ np.asarray(Wo, np.float32).T + np.asarray(bo, np.float32)


def run_cores(budgets, in_maps, trace=False, **kw):
    """Run the compiled program on cores 0-7."""
    from concourse.bass_utils import run_bass_kernel_spmd

    nc = build_program(budgets)
    return run_bass_kernel_spmd(nc, in_maps, list(range(N_CORES)),
                                trace=trace, **kw)


def kernel(X, Wq, bq, Wk, bk, Wv, bv, Wo, bo, valid_lens):
    if np.any(np.asarray(bq)) or np.any(np.asarray(bk)):
        # never the case for this problem's setup_inputs (zeros);
        # exact fallback kept for safety.
        return _numpy_fallback(X, Wq, bq, Wk, bk, Wv, bv, Wo, bo, valid_lens)
    budgets, in_maps = make_in_maps(X, Wq, Wk, Wv, Wo, valid_lens)
    res = run_cores(budgets, in_maps, trace=False)
    return assemble(res.results, Wo, bv, bo)
